# revision 44
# baseline (speedup 1.0000x reference)
"""Bass/Trainium2 kernel for 2-layer GAT (nn_GAT_58128087384143).

Strategy (8 NeuronCores, one SPMD NEFF):
  - Destination nodes are partitioned across the 8 cores, degree-sorted and
    assigned round-robin by rank so every core's tile t holds similarly
    sized ELL rows; one shared (core, slot) layout serves both layers'
    segment softmax / aggregation.
  - Every core computes the full "hext" node table (replicated):
    hext[row(n)] = [h(n) as bf16 | al_src(n)] where h = x @ W and
    al_src = x @ (W a_src) come out of one PE matmul per 128-node tile
    (phase A), stored p-major so writes are a few large descriptors.
  - al_dst is never gathered: each core computes it for its own dst slots
    with tiny PE matmuls against xS (host-permuted x, layer 1) or its own
    relu(out1).T slice (layer 2).
  - Per layer, per dst-tile group (phase C): dma_gather fetches the 256-byte
    hext rows of all in-edge sources (degree-bucketed ELL, padded slots point
    at sentinel rows whose al = -1e30 so exp() kills them), ACT computes
    leaky_relu logits (Lrelu) + exp with a fused row-sum (no max-subtraction:
    logits are provably < 15 for this model family), DVE does the broadcast
    multiply + k-reduction, ACT applies 1/denom, DVE adds bias.
  - dma_gather indices are int16, so each hext table is addressed through
    two 32768-row windows; edges whose source row lands in the overlap go to
    whichever side minimizes the per-tile ELL widths.
  - Between layers, relu(out1).T is AllGathered in column chunks so the
    collective overlaps the tail of phase C1 and hext2 construction (phase
    A2) proceeds per-chunk as data arrives.

kernel(**inputs) -> np.ndarray [50000, 64] float32.
"""

import numpy as np

P = 128
NCORES = 8
N = 50000
F_IN = 128
HID = 64
T = 49                 # dst tiles per core
S = T * P              # 6272 dst slots per core
CONCAT = NCORES * S    # 50176
NR1 = N + 3            # rows: 0=sent_neg, 1=sent_zero, 2..N+1 nodes, N+2=sent_neg_hi
NR2 = CONCAT + 3
WIN = 32768            # int16 gather window
HIB1 = NR1 - WIN       # hi window base row
HIB2 = NR2 - WIN
ROWW = 64              # fp32 elements per hext row (256 B); h is bf16
ALS = 32               # hext f32 col of al_src
ALD = 33               # hext f32 col of al_dst (written/used by layer 2 only)
NEGINF = -1.0e30
NEG_SLOPE = 0.2
RHSW = 128             # phase-A rhs width: [W | wa_src | wa_dst | 0...]
CHUNK = 4096           # phase-A input streaming chunk (cols)
STAGE_TILES = 32       # node-tiles per hext store
KCAP = 48              # max summed ELL width per gather group
CSPLIT = (12, 13, 14, 10)  # collective chunks, in dst tiles (small tail)
FP8_EXCHANGE = True    # ship relu(out1) as fp8 e4m3 instead of bf16


# ----------------------------------------------------------------------------
# host-side graph preprocessing
# ----------------------------------------------------------------------------

def _cumcount(keys_sorted):
    n = keys_sorted.shape[0]
    if n == 0:
        return np.zeros(0, np.int64)
    first = np.ones(n, bool)
    first[1:] = keys_sorted[1:] != keys_sorted[:-1]
    idx = np.arange(n)
    start = np.maximum.accumulate(np.where(first, idx, 0))
    return idx - start


def _pack16(flat):
    """[n] int -> [128, n//16] int16: idx j at partition j%16, col j//16,
    replicated 8x down the partitions (one copy per Q7 core pair)."""
    n = flat.shape[0]
    assert n % 16 == 0
    block = flat.reshape(n // 16, 16).T.astype(np.int16)
    return np.tile(block, (8, 1))


def _window_k(mustlo, musthi, deg):
    """Per-tile optimal ELL widths given per-slot must-lo/must-hi/total
    degrees shaped [NCORES, T, P]. Returns K_lo[T], K_hi[T]."""
    A = mustlo.max(axis=(0, 2))
    B = musthi.max(axis=(0, 2))
    D = deg.max(axis=(0, 2))
    K_lo = np.maximum(A, 1)   # >=1 so dead slots have a sentinel column
    K_hi = np.maximum(B, D - K_lo)
    K_hi = np.maximum(K_hi, 0)
    return K_lo.astype(np.int64), K_hi.astype(np.int64)


def _order_score(key_cols, mustlo_d, musthi_d, deg_d):
    order = np.lexsort(key_cols)
    ml = np.zeros(CONCAT, np.int64)
    mh = np.zeros(CONCAT, np.int64)
    dg = np.zeros(CONCAT, np.int64)
    ml[:N] = mustlo_d[order]
    mh[:N] = musthi_d[order]
    dg[:N] = deg_d[order]
    ml = ml.reshape(S, NCORES).T.reshape(NCORES, T, P)
    mh = mh.reshape(S, NCORES).T.reshape(NCORES, T, P)
    dg = dg.reshape(S, NCORES).T.reshape(NCORES, T, P)
    K_lo, K_hi = _window_k(ml, mh, dg)
    return (K_lo + K_hi).sum() * P * NCORES, order


def _side_assign(dst, mustlo_e, musthi_e, K_lo_of_dst, K_hi_of_dst,
                 deg_d, mustlo_d):
    """Choose lo/hi side per edge (flex edges fill lo up to what K_hi can't
    absorb)."""
    flex_e = ~(mustlo_e | musthi_e)
    lo_cap = K_lo_of_dst - mustlo_d
    need_lo = deg_d - mustlo_d - K_hi_of_dst
    x_d = np.clip(need_lo, 0, np.maximum(lo_cap, 0))
    order = np.lexsort((~flex_e, dst))
    pos = _cumcount(dst[order])
    flexrank = np.full(dst.shape[0], 1 << 30, np.int64)
    flexrank[order] = np.where(flex_e[order], pos, 1 << 30)
    lo_e = mustlo_e | (flexrank < x_d[dst])
    return lo_e


def _make_groups(K_lo, K_hi, cap, csplit):
    """Greedy grouping of dst tiles into gather groups with summed width
    <= cap, never straddling collective-chunk boundaries (layer 1 uses the
    boundaries so chunk j's tiles finish before its AllGather; layer 2 just
    reuses the same grouping code with one big chunk)."""
    bounds = []
    t0 = 0
    for c in csplit:
        bounds.append((t0, t0 + c))
        t0 += c
    groups = []
    for (b0, b1) in bounds:
        t = b0
        while t < b1:
            kp = int(K_lo[t] + K_hi[t])
            ts = [t]
            t += 1
            while t < b1 and kp + int(K_lo[t] + K_hi[t]) <= cap:
                kp += int(K_lo[t] + K_hi[t])
                ts.append(t)
                t += 1
            groups.append(ts)
    return groups


def _build_ell(dst, row_of_edge, lo_e, core_of_dst, pos_of_dst,
               K_lo, K_hi, hib, sent_hi_val, dead, groups):
    """Build per-core packed int16 index arrays for the per-group gathers.

    Group g covers tiles ts: one lo index block (tiles concatenated k-major)
    then one hi block, so each group needs two dma_gather calls."""
    core_e = core_of_dst[dst]
    pos_e = pos_of_dst[dst]
    side_e = (~lo_e).astype(np.int64)
    order = np.lexsort((side_e, pos_e, core_e))
    key = ((core_e[order] * S + pos_e[order]) << 1) | side_e[order]
    cc = _cumcount(key)

    KLM = int(K_lo.max())
    KHM = int(max(1, K_hi.max()))
    ell_lo = np.zeros((NCORES, S, KLM), np.int64)            # sent_neg = row 0
    ell_hi = np.full((NCORES, S, KHM), sent_hi_val, np.int64)
    oe = order
    lo_sel = lo_e[oe]
    ell_lo[core_e[oe][lo_sel], pos_e[oe][lo_sel], cc[lo_sel]] = \
        row_of_edge[oe][lo_sel]
    hi_sel = ~lo_sel
    ell_hi[core_e[oe][hi_sel], pos_e[oe][hi_sel], cc[hi_sel]] = \
        row_of_edge[oe][hi_sel] - hib
    # dead slots: first lo column -> sent_zero (row 1) so denom = 1, out = 0
    dc, dp = np.nonzero(dead)
    ell_lo[dc, dp, 0] = 1

    packs = [[] for _ in range(NCORES)]
    ginfo = []   # (idx_lo_off, n_lo, idx_hi_off, n_hi, kp) per group
    tinfo = []   # (group, lo_off, kl, hi_off, kh) per tile, tile-indexed
    col = 0
    tmap = {}
    for gi, ts in enumerate(groups):
        kls = [int(K_lo[t]) for t in ts]
        khs = [int(K_hi[t]) for t in ts]
        n_lo, n_hi = sum(kls), sum(khs)
        kp = n_lo + n_hi
        lo_off = col
        col += 8 * n_lo
        hi_off = col
        col += 8 * n_hi
        ginfo.append((lo_off, n_lo, hi_off, n_hi, kp))
        run = 0
        for i, t in enumerate(ts):
            tmap[t] = (gi, run, kls[i], n_lo + sum(khs[:i]), khs[i])
            run += kls[i]
        for c in range(NCORES):
            blks = [ell_lo[c, t * P:(t + 1) * P, :int(K_lo[t])].T.reshape(-1)
                    for t in ts]
            packs[c].append(_pack16(np.concatenate(blks)))
            if n_hi:
                blks = [ell_hi[c, t * P:(t + 1) * P, :int(K_hi[t])].T.reshape(-1)
                        for t in ts]
                packs[c].append(_pack16(np.concatenate(blks)))
    tinfo = [tmap[t] for t in range(T)]
    idx = np.stack([np.concatenate(p, axis=1) for p in packs])  # [NC,128,C]
    return np.ascontiguousarray(idx), (ginfo, tinfo, groups), col


def _rowmap_pmajor(total):
    """DRAM row offset for each sequential stream position, matching phase
    A1's p-major stage stores over CHUNK-column spans."""
    rm = np.empty(total, np.int64)
    for base in range(0, total, CHUNK):
        cols = min(CHUNK, total - base)
        idx = np.arange(cols)
        if cols % P == 0:
            ntile = cols // P
            rm[base:base + cols] = base + (idx % P) * ntile + idx // P
        else:
            rm[base:base + cols] = base + idx
    return rm


def _rowmap_csplit():
    """Per-core row offset of each slot for phase A2's per-collective-chunk
    p-major stores."""
    rm = np.empty(S, np.int64)
    base = 0
    for c in CSPLIT:
        cols = c * P
        idx = np.arange(cols)
        rm[base:base + cols] = base + (idx % P) * c + idx // P
        base += cols
    return rm


def _preprocess(edge_index):
    src = np.concatenate([edge_index[0].astype(np.int64), np.arange(N)])
    dst = np.concatenate([edge_index[1].astype(np.int64), np.arange(N)])
    deg_d = np.bincount(dst, minlength=N)
    outdeg = np.bincount(src, minlength=N)

    # ---------- layer-1 row placement (via host x permutation) ----------
    # top-out-degree nodes go to rows in the two-window overlap so the most
    # edges become side-flexible; the rest alternate lo/hi by out-degree.
    rows_of_pos = _rowmap_pmajor(N) + 2
    ov = (rows_of_pos >= HIB1) & (rows_of_pos < WIN)
    over_pos = np.where(ov)[0]
    lo_pos = np.where(rows_of_pos < HIB1)[0]
    hi_pos = np.where(rows_of_pos >= WIN)[0]
    by_out = np.argsort(-outdeg, kind="stable")
    pos_of_node = np.empty(N, np.int64)
    pos_of_node[by_out[:over_pos.size]] = over_pos
    rest = by_out[over_pos.size:]
    nlo, nhi = lo_pos.size, hi_pos.size
    sel_lo, sel_hi = [], []
    li = hi = 0
    for i, n in enumerate(rest):
        if (i % 2 == 0 and li < nlo) or hi >= nhi:
            sel_lo.append(n); li += 1
        else:
            sel_hi.append(n); hi += 1
    pos_of_node[np.array(sel_lo)] = lo_pos
    pos_of_node[np.array(sel_hi)] = hi_pos
    xposinv = np.empty(N, np.int64)
    xposinv[pos_of_node] = np.arange(N)      # stream position -> node
    rowmap1 = rows_of_pos[pos_of_node]       # hext1 row of node
    row1 = rowmap1[src]
    mustlo1_e = row1 < HIB1
    musthi1_e = row1 >= WIN
    mustlo1_d = np.bincount(dst[mustlo1_e], minlength=N)
    musthi1_d = np.bincount(dst[musthi1_e], minlength=N)

    blk_map = _rowmap_csplit()
    cands = [
        (-deg_d, -mustlo1_d),
        (-musthi1_d, -mustlo1_d),
        (-mustlo1_d, -musthi1_d),
        (-(mustlo1_d + musthi1_d), -deg_d),
        (-deg_d, -(mustlo1_d - musthi1_d)),
        (-np.maximum(mustlo1_d, musthi1_d), -deg_d),
    ]
    best = None
    for kc in cands:
        score, order = _order_score(kc, mustlo1_d, musthi1_d, deg_d)
        if best is None or score < best[0]:
            best = (score, order)
    slots1, order1 = best
    rank1 = np.empty(N, np.int64)
    rank1[order1] = np.arange(N)
    core_of = rank1 % NCORES
    pos = rank1 // NCORES
    dead = np.ones((NCORES, S), np.uint8)
    dead[core_of, pos] = 0

    ml = np.zeros(CONCAT, np.int64); mh = np.zeros(CONCAT, np.int64)
    dg = np.zeros(CONCAT, np.int64)
    ml[:N] = mustlo1_d[order1]; mh[:N] = musthi1_d[order1]
    dg[:N] = deg_d[order1]
    K1_lo, K1_hi = _window_k(ml.reshape(S, NCORES).T.reshape(NCORES, T, P),
                             mh.reshape(S, NCORES).T.reshape(NCORES, T, P),
                             dg.reshape(S, NCORES).T.reshape(NCORES, T, P))
    lo1_e = _side_assign(dst, mustlo1_e, musthi1_e, K1_lo[pos // P],
                         K1_hi[pos // P], deg_d, mustlo1_d)
    groups1 = _make_groups(K1_lo, K1_hi, KCAP, CSPLIT)
    idx1, offs1, C1 = _build_ell(dst, row1, lo1_e, core_of, pos,
                                 K1_lo, K1_hi, HIB1, NR1 - 1 - HIB1, dead,
                                 groups1)

    # ---------- layer 2 (own dst ordering; source rows are concat slots) --
    crow = core_of * S + pos
    rowmap2cat = ((crow // S) * S + blk_map[crow % S]) + 2
    r2 = rowmap2cat[src]
    mustlo2_e = r2 < HIB2
    musthi2_e = r2 >= WIN
    mustlo2_d = np.bincount(dst[mustlo2_e], minlength=N)
    musthi2_d = np.bincount(dst[musthi2_e], minlength=N)

    cands2 = [
        (-deg_d, -mustlo2_d, core_of),
        (-musthi2_d, -mustlo2_d, core_of),
        (-mustlo2_d, -musthi2_d, core_of),
        (-deg_d, -(mustlo2_d - musthi2_d), core_of),
        (-(mustlo2_d + musthi2_d), -deg_d, core_of),
        (-np.maximum(mustlo2_d, musthi2_d), -deg_d, core_of),
    ]
    best2 = None
    for kc in cands2:
        o2 = np.lexsort(kc)
        p2 = np.empty(N, np.int64)
        p2[o2] = _cumcount(core_of[o2])
        ml = np.zeros((NCORES, S), np.int64)
        mh = np.zeros((NCORES, S), np.int64)
        dg2 = np.zeros((NCORES, S), np.int64)
        ml[core_of, p2] = mustlo2_d
        mh[core_of, p2] = musthi2_d
        dg2[core_of, p2] = deg_d
        klo, khi = _window_k(ml.reshape(NCORES, T, P),
                             mh.reshape(NCORES, T, P),
                             dg2.reshape(NCORES, T, P))
        score = int((klo + khi).sum())
        if best2 is None or score < best2[0]:
            best2 = (score, p2, klo, khi)
    _, pos2, K2_lo, K2_hi = best2
    slots2 = int((K2_lo + K2_hi).sum()) * P * NCORES
    dead2 = np.ones((NCORES, S), np.uint8)
    dead2[core_of, pos2] = 0

    lo2_e = _side_assign(dst, mustlo2_e, musthi2_e, K2_lo[pos2 // P],
                         K2_hi[pos2 // P], deg_d, mustlo2_d)
    groups2 = _make_groups(K2_lo, K2_hi, KCAP, (T,))
    idx2, offs2, C2 = _build_ell(dst, r2, lo2_e, core_of, pos2,
                                 K2_lo, K2_hi, HIB2, NR2 - 1 - HIB2, dead2,
                                 groups2)

    # ---------- per-dst-row al_dst2 gather (layer 2 phase B) ----------
    rect_lo = np.zeros((NCORES, S), np.int64)
    rect_hi = np.full((NCORES, S), NR2 - 1 - HIB2, np.int64)
    mask_hi = np.zeros((NCORES, S), np.uint8)
    r = rowmap2cat
    is_lo = r < WIN
    rect_lo[core_of[is_lo], pos2[is_lo]] = r[is_lo]
    ih = ~is_lo
    rect_hi[core_of[ih], pos2[ih]] = r[ih] - HIB2
    mask_hi[core_of[ih], pos2[ih]] = 1
    # dead slots: lo sentinel-zero row so al_dst = 0
    rect_lo[dead2 > 0] = 1
    didx2 = np.stack([np.concatenate(
        [_pack16(rect_lo[c]), _pack16(rect_hi[c])], axis=1)
        for c in range(NCORES)])
    mh2 = np.ascontiguousarray(np.stack(
        [mask_hi[c].reshape(T, P).T for c in range(NCORES)]))

    stats = dict(slots1=int(slots1), slots2=int(slots2),
                 edges=int(dst.shape[0]),
                 pad1=float(slots1) / dst.shape[0],
                 pad2=float(slots2) / dst.shape[0])
    return dict(idx1=idx1, offs1=offs1, C1=C1, K1_lo=K1_lo, K1_hi=K1_hi,
                rowmap1=rowmap1, rowmap2cat=rowmap2cat, crow=crow,
                xposinv=xposinv,
                idx2=idx2, offs2=offs2, C2=C2, K2_lo=K2_lo, K2_hi=K2_hi,
                didx2=didx2, mh2=mh2,
                core_of=core_of, pos=pos, pos2=pos2, blk_map=blk_map,
                stats=stats)


# ----------------------------------------------------------------------------
# device kernel
# ----------------------------------------------------------------------------

def _build_nc(pre):
    import concourse.bass as bass
    import concourse.mybir as mybir
    import concourse.tile as tile
    from concourse import bacc
    from concourse.masks import make_identity

    f32 = mybir.dt.float32
    bf16 = mybir.dt.bfloat16
    i16 = mybir.dt.int16
    x8 = mybir.dt.float8e4 if FP8_EXCHANGE else bf16
    AF = mybir.ActivationFunctionType
    OP = mybir.AluOpType
    AX = mybir.AxisListType

    offs1, offs2 = pre["offs1"], pre["offs2"]
    C1, C2 = pre["C1"], pre["C2"]

    nc = bacc.Bacc("TRN2", num_devices=NCORES, target_bir_lowering=False)

    xT = nc.dram_tensor("xT", [F_IN, N], bf16, kind="ExternalInput")
    xS = nc.dram_tensor("xS", [F_IN, S], bf16, kind="ExternalInput")
    rhs1 = nc.dram_tensor("rhs1", [F_IN, RHSW], f32, kind="ExternalInput")
    rhs2 = nc.dram_tensor("rhs2", [HID, RHSW], f32, kind="ExternalInput")
    wad1 = nc.dram_tensor("wad1", [F_IN, 1], bf16, kind="ExternalInput")
    b1r = nc.dram_tensor("b1r", [P, HID], f32, kind="ExternalInput")
    b2r = nc.dram_tensor("b2r", [P, HID], f32, kind="ExternalInput")
    idx1 = nc.dram_tensor("idx1", [P, C1], i16, kind="ExternalInput")
    idx2 = nc.dram_tensor("idx2", [P, C2], i16, kind="ExternalInput")
    didx2 = nc.dram_tensor("didx2", [P, 2 * (S // 16)], i16,
                           kind="ExternalInput")
    mh2 = nc.dram_tensor("mh2", [P, T], mybir.dt.uint8, kind="ExternalInput")
    out2 = nc.dram_tensor("out", [S, HID], f32, kind="ExternalOutput")

    hext1 = nc.dram_tensor("hext1", [NR1, ROWW], f32, kind="Internal")
    hext2 = nc.dram_tensor("hext2", [NR2, ROWW], f32, kind="Internal")
    o1c = [nc.dram_tensor(f"o1c{j}", [HID, CSPLIT[j] * P], x8, kind="Internal")
           for j in range(len(CSPLIT))]
    ag = [nc.dram_tensor(f"ag{j}", [NCORES, HID, CSPLIT[j] * P], x8,
                         kind="Internal", addr_space="Shared")
          for j in range(len(CSPLIT))]

    KMAX = int(max(max(g[4] for g in offs1[0]), max(g[4] for g in offs2[0])))
    NTGMAX = max(max(len(ts) for ts in offs1[2]),
                 max(len(ts) for ts in offs2[2]))

    with tile.TileContext(nc) as tc:
        with tc.tile_pool(name="const", bufs=1) as cp:
            rhs1_sb = cp.tile([F_IN, RHSW], bf16)
            nc.gpsimd.dma_start(out=rhs1_sb[:], in_=rhs1[:, :])
            rhs2_sb = cp.tile([HID, RHSW], bf16)
            nc.gpsimd.dma_start(out=rhs2_sb[:], in_=rhs2[:, :])
            wad1_sb = cp.tile([F_IN, 1], bf16)
            nc.gpsimd.dma_start(out=wad1_sb[:], in_=wad1[:, :])
            b1_sb = cp.tile([P, HID], f32)
            nc.sync.dma_start(out=b1_sb[:], in_=b1r[:, :])
            b2_sb = cp.tile([P, HID], f32)
            nc.sync.dma_start(out=b2_sb[:], in_=b2r[:, :])
            ident = cp.tile([P, P], f32)
            make_identity(nc, ident[:])
            xS_sb = cp.tile([F_IN, S], bf16)
            nc.sync.dma_start(out=xS_sb[:], in_=xS[:, :])
            o1T_sb = cp.tile([HID, S], bf16)
            ald1 = cp.tile([P, T], f32)
            ald1_02 = cp.tile([P, T], f32)
            # sentinel rows: row0 al=-1e30 (pad), row1 al=0 (dead slots)
            sent = cp.tile([2, ROWW], f32)
            nc.vector.memset(sent[:], 0.0)
            nc.vector.memset(sent[0:1, ALS:ALS + 1], NEGINF)
            # index tables: load up front so they never queue behind
            # collective-gated DMAs
            idx1_sb = cp.tile([P, C1], i16)
            nc.sync.dma_start(out=idx1_sb[:], in_=idx1[:, :])
            idx2_sb = cp.tile([P, C2], i16)
            nc.sync.dma_start(out=idx2_sb[:], in_=idx2[:, :])
            didx2_sb = cp.tile([P, 2 * (S // 16)], i16)
            nc.sync.dma_start(out=didx2_sb[:], in_=didx2[:, :])
            mh2_sb = cp.tile([P, T], mybir.dt.uint8)
            nc.sync.dma_start(out=mh2_sb[:], in_=mh2[:, :])

            def phase_b1():
                """al_dst1 for this core's own dst slots via tiny matmuls
                against the host-permuted x (no gather needed)."""
                with tc.tile_pool(name="pb1", bufs=1, space="PSUM") as pb:
                    ps = pb.tile([P, T], f32, space="PSUM")
                    for t in range(T):
                        nc.tensor.matmul(out=ps[:, t:t + 1],
                                         lhsT=xS_sb[:, t * P:(t + 1) * P],
                                         rhs=wad1_sb[:],
                                         start=True, stop=True,
                                         skip_group_check=True)
                    nc.scalar.activation(out=ald1[:], in_=ps[:], func=AF.Copy)
                    nc.vector.tensor_scalar(out=ald1_02[:], in0=ald1[:],
                                            scalar1=NEG_SLOPE, scalar2=None,
                                            op0=OP.mult)

            def phase_a(layer):
                assert layer == 1
                hext = hext1
                rhs_sb = rhs1_sb
                kdim = F_IN
                with tc.tile_pool(name=f"pa{layer}", bufs=3) as pa, \
                     tc.tile_pool(name=f"pap{layer}", bufs=6, space="PSUM") as pp:
                    nc.sync.dma_start(out=hext[0:2, :], in_=sent[:])
                    nc.sync.dma_start(out=hext[NR1 - 1:NR1, :],
                                      in_=sent[0:1, :])

                    spans = [(c0, min(CHUNK, N - c0), 0, 2 + c0)
                             for c0 in range(0, N, CHUNK)]
                    for c0, cols, blk, rowbase in spans:
                        in_sb = pa.tile([kdim, CHUNK], bf16, tag="pa_in")
                        nc.sync.dma_start(out=in_sb[:, 0:cols],
                                          in_=xT[:, c0:c0 + cols])
                        ntile = (cols + P - 1) // P
                        stage = pa.tile([P, STAGE_TILES, ROWW], f32, tag="pa_st")
                        stage_bf = stage[:].bitcast(bf16)
                        QUAD = 4
                        nt = 0
                        while nt < ntile:
                            q = min(QUAD, ntile - nt)
                            rows = [min(P, cols - (nt + i) * P)
                                    for i in range(q)]
                            if rows[0] == P:
                                while q > 1 and rows[q - 1] < P:
                                    q -= 1
                            else:
                                q = 1
                            r = rows[0] if q == 1 else P
                            ps = pp.tile([P, QUAD, RHSW], f32, space="PSUM")
                            for i in range(q):
                                nc.tensor.matmul(
                                    out=ps[0:r, i, :],
                                    lhsT=in_sb[:, (nt + i) * P:
                                               (nt + i) * P + r],
                                    rhs=rhs_sb[:],
                                    start=True, stop=True,
                                    skip_group_check=True)
                            nc.scalar.activation(
                                out=stage_bf[0:r, nt:nt + q, 0:HID],
                                in_=ps[0:r, 0:q, 0:HID], func=AF.Copy)
                            nc.vector.tensor_copy(
                                out=stage[0:r, nt:nt + q, ALS:ALS + 1],
                                in_=ps[0:r, 0:q, HID:HID + 1])
                            nt += q
                        full = cols // P
                        rem = cols - full * P
                        if rem == 0:
                            nc.sync.dma_start(
                                out=hext[rowbase:rowbase + cols, :]
                                .rearrange("(p n) w -> p n w", p=P),
                                in_=stage[:, 0:full, :])
                        else:
                            if full:
                                nc.sync.dma_start(
                                    out=hext[rowbase:rowbase + full * P, :]
                                    .rearrange("(n p) w -> p n w", p=P),
                                    in_=stage[:, 0:full, :])
                            nc.sync.dma_start(
                                out=hext[rowbase + full * P:
                                         rowbase + full * P + rem, :]
                                .rearrange("(n p) w -> p n w", p=rem),
                                in_=stage[0:rem, full:full + 1, :])

            def phase_c(layer, bc, gp, b1p, bp, on_chunk_done=None):
                hext = hext1 if layer == 1 else hext2
                hib = HIB1 if layer == 1 else HIB2
                offs = offs1 if layer == 1 else offs2
                idx_t = idx1 if layer == 1 else idx2
                cdim = C1 if layer == 1 else C2
                b_sb = b1_sb if layer == 1 else b2_sb

                src_lo = hext[0:WIN, :]
                src_hi = hext[hib:hib + WIN, :]
                ginfo, tinfo, groups = offs

                idx_sb = idx1_sb if layer == 1 else idx2_sb
                if layer == 1:
                    ald = ald1
                    ald02 = ald1_02
                else:
                    o2_sb = b1p.tile([P, T, HID], f32)
                    # al_dst2 per own dst slot: gather hext2 rows, read ALD
                    Gd_lo = b1p.tile([P, T, ROWW], f32)
                    nc.gpsimd.dma_gather(
                        out_ap=Gd_lo[:], in_ap=src_lo,
                        idxs_ap=didx2_sb[:, 0:S // 16],
                        num_idxs=S, num_idxs_reg=S, elem_size=ROWW,
                        single_packet=False)
                    Gd_hi = b1p.tile([P, T, ROWW], f32)
                    nc.gpsimd.dma_gather(
                        out_ap=Gd_hi[:], in_ap=src_hi,
                        idxs_ap=didx2_sb[:, S // 16:2 * (S // 16)],
                        num_idxs=S, num_idxs_reg=S, elem_size=ROWW,
                        single_packet=False)
                    ald = b1p.tile([P, T], f32)
                    nc.vector.tensor_copy(out=ald[:], in_=Gd_lo[:, :, ALD])
                    nc.vector.copy_predicated(out=ald[:], mask=mh2_sb[:],
                                              data=Gd_hi[:, :, ALD])
                    ald02 = b1p.tile([P, T], f32)
                    nc.vector.tensor_scalar(out=ald02[:], in0=ald[:],
                                            scalar1=NEG_SLOPE, scalar2=None,
                                            op0=OP.mult)

                chunk_end = []
                t0 = 0
                for c in CSPLIT:
                    chunk_end.append(t0 + c)
                    t0 += c
                pending = []

                def emit_exchange(j):
                    nc.gpsimd.collective_compute(
                        kind="AllGather", op=OP.bypass,
                        replica_groups=[list(range(NCORES))],
                        ins=[o1c[j][:, :]], outs=[ag[j][:, :, :]])
                    # hext2 build for the PREVIOUS chunk: its collective has
                    # landed by now, so the SP queue never parks on an
                    # unfinished AllGather in front of later o1c stores.
                    if on_chunk_done is not None and j > 0:
                        on_chunk_done(j - 1)

                for gi, ts in enumerate(groups):
                    ilo, n_lo, ihi, n_hi, kp = ginfo[gi]
                    G = gp.tile([P, KMAX, ROWW], f32, tag="G")
                    nc.gpsimd.dma_gather(
                        out_ap=G[:, 0:n_lo, :], in_ap=src_lo,
                        idxs_ap=idx_sb[:, ilo:ilo + 8 * n_lo],
                        num_idxs=P * n_lo, num_idxs_reg=P * n_lo,
                        elem_size=ROWW, single_packet=False)
                    if n_hi:
                        nc.gpsimd.dma_gather(
                            out_ap=G[:, n_lo:kp, :], in_ap=src_hi,
                            idxs_ap=idx_sb[:, ihi:ihi + 8 * n_hi],
                            num_idxs=P * n_hi, num_idxs_reg=P * n_hi,
                            elem_size=ROWW, single_packet=False)
                    Gh = G[:].bitcast(bf16)
                    ntg = len(ts)
                    exg = bc.tile([P, KMAX], f32, tag="exg")
                    e0 = bc.tile([P, KMAX], f32, tag="e0")
                    e1 = bc.tile([P, KMAX], f32, tag="e1")
                    den = bc.tile([P, 2, NTGMAX], f32, tag="den")
                    rec = bc.tile([P, NTGMAX], f32, tag="rec")
                    any_hi = any(tinfo[t][4] for t in ts)
                    if any_hi:
                        nc.vector.memset(den[:, 1, 0:ntg], 0.0)
                    reds = []
                    # pass 1: ex = exp(leaky_relu(al_src + al_dst)) in group
                    # layout; per-range row-sums -> den.  No max subtraction:
                    # logits are bounded (~15) for this model family.
                    for ti, t in enumerate(ts):
                        _, lo_off, kl, hi_off, kh = tinfo[t]
                        ad = ald[:, t:t + 1]
                        ad02 = ald02[:, t:t + 1]
                        for si, (o, k) in enumerate(((lo_off, kl),
                                                     (hi_off, kh))):
                            if k == 0:
                                continue
                            # leaky_relu(x + ad) = max(x + ad, 0.2x + 0.2ad)
                            nc.scalar.activation(
                                out=e0[:, o:o + k], in_=G[:, o:o + k, ALS],
                                func=AF.Identity, bias=ad, scale=1.0)
                            nc.scalar.activation(
                                out=e1[:, o:o + k], in_=G[:, o:o + k, ALS],
                                func=AF.Identity, bias=ad02,
                                scale=NEG_SLOPE)
                            nc.vector.tensor_tensor(
                                out=e1[:, o:o + k], in0=e0[:, o:o + k],
                                in1=e1[:, o:o + k], op=OP.max)
                            nc.scalar.activation(
                                out=exg[:, o:o + k], in_=e1[:, o:o + k],
                                func=AF.Exp,
                                accum_out=den[:, si, ti:ti + 1])
                        # weighted sum can start before the denominators are
                        # merged — only the final scale needs 1/den
                        kt = kl + kh
                        prod = bc.tile([P, KMAX, HID], bf16, tag="prod")
                        for (o, k, d0) in ((lo_off, kl, 0),
                                           (hi_off, kh, kl)):
                            if k == 0:
                                continue
                            nc.vector.tensor_tensor(
                                out=prod[:, d0:d0 + k, :],
                                in0=Gh[:, o:o + k, 0:HID],
                                in1=exg[:, o:o + k, None]
                                .to_broadcast([P, k, HID]),
                                op=OP.mult)
                        red = bc.tile([P, HID], f32, tag=f"red{ti}")
                        nc.vector.tensor_reduce(
                            out=red[:], in_=prod[:, 0:kt, :].rearrange(
                                "p k f -> p f k"),
                            axis=AX.X, op=OP.add)
                        reds.append(red)
                    if any_hi:
                        nc.vector.tensor_tensor(out=den[:, 0, 0:ntg],
                                                in0=den[:, 0, 0:ntg],
                                                in1=den[:, 1, 0:ntg],
                                                op=OP.add)
                    nc.vector.reciprocal(out=rec[:, 0:ntg],
                                         in_=den[:, 0, 0:ntg])
                    # pass 2: normalize + bias + store per tile
                    for ti, t in enumerate(ts):
                        outt = bc.tile([P, HID], f32, tag="outt")
                        nc.scalar.activation(out=outt[:], in_=reds[ti][:],
                                             func=AF.Copy,
                                             scale=rec[:, ti:ti + 1])
                        if layer == 1:
                            nc.vector.tensor_tensor(out=outt[:], in0=outt[:],
                                                    in1=b_sb[:], op=OP.add)
                            psT = bp.tile([HID, P], f32, space="PSUM")
                            nc.tensor.transpose(out=psT[:], in_=outt[:],
                                                identity=ident[:])
                            nc.scalar.activation(
                                out=o1T_sb[:, t * P:(t + 1) * P],
                                in_=psT[:], func=AF.Relu)
                        else:
                            nc.vector.tensor_tensor(out=o2_sb[:, t, :],
                                                    in0=outt[:], in1=b_sb[:],
                                                    op=OP.add)
                    if layer == 1 and ts[-1] + 1 in chunk_end:
                        j = chunk_end.index(ts[-1] + 1)
                        cbase = (chunk_end[j - 1] if j else 0) * P
                        cw = CSPLIT[j] * P
                        if FP8_EXCHANGE:
                            o1x = b1p.tile([HID, S], x8, tag="o1x")
                            nc.vector.tensor_copy(
                                out=o1x[:, cbase:cbase + cw],
                                in_=o1T_sb[:, cbase:cbase + cw])
                            nc.sync.dma_start(out=o1c[j][:, :],
                                              in_=o1x[:, cbase:cbase + cw])
                        else:
                            nc.sync.dma_start(out=o1c[j][:, :],
                                              in_=o1T_sb[:, cbase:cbase + cw])
                        # the collective itself is emitted one group later:
                        # it parks Pool SEQ until o1c lands, so give the next
                        # chunk's gather preps a head start in the queue
                        pending.append((gi + 1, j))
                    while pending and pending[0][0] <= gi:
                        emit_exchange(pending.pop(0)[1])

                while pending:
                    emit_exchange(pending.pop(0)[1])
                if layer == 1 and on_chunk_done is not None:
                    on_chunk_done(len(CSPLIT) - 1)

                if layer == 2:
                    nc.sync.dma_start(
                        out=out2[:, :].rearrange("(p t) f -> p t f", p=P),
                        in_=o2_sb[:])

            def phase_a2_chunk(j, pa, pp):
                """hext2 rows for collective chunk j, all 8 source blocks."""
                cbase = sum(CSPLIT[:j]) * P
                cols = CSPLIT[j] * P
                ntile = CSPLIT[j]
                for blk in range(NCORES):
                    rowbase = 2 + blk * S + cbase
                    in_sb = pa.tile([HID, CHUNK], bf16, tag="pa_in")
                    if FP8_EXCHANGE:
                        raw = pa.tile([HID, CHUNK], x8, tag="pa_raw")
                        nc.sync.dma_start(out=raw[:, 0:cols],
                                          in_=ag[j][blk, :, :])
                        nc.vector.tensor_copy(out=in_sb[:, 0:cols],
                                              in_=raw[:, 0:cols])
                    else:
                        nc.sync.dma_start(out=in_sb[:, 0:cols],
                                          in_=ag[j][blk, :, :])
                    stage = pa.tile([P, STAGE_TILES, ROWW], f32, tag="pa_st")
                    stage_bf = stage[:].bitcast(bf16)
                    QUAD = 4
                    nt = 0
                    while nt < ntile:
                        q = min(QUAD, ntile - nt)
                        ps = pp.tile([P, QUAD, RHSW], f32, space="PSUM")
                        for i in range(q):
                            nc.tensor.matmul(
                                out=ps[:, i, :],
                                lhsT=in_sb[:, (nt + i) * P:(nt + i + 1) * P],
                                rhs=rhs2_sb[:],
                                start=True, stop=True,
                                skip_group_check=True)
                        nc.scalar.activation(
                            out=stage_bf[:, nt:nt + q, 0:HID],
                            in_=ps[:, 0:q, 0:HID], func=AF.Copy)
                        nc.vector.tensor_copy(
                            out=stage[:, nt:nt + q, ALS:ALD + 1],
                            in_=ps[:, 0:q, HID:HID + 2])
                        nt += q
                    nc.sync.dma_start(
                        out=hext2[rowbase:rowbase + cols, :]
                        .rearrange("(p n) w -> p n w", p=P),
                        in_=stage[:, 0:ntile, :])

            phase_a(1)
            phase_b1()
            # layer-1 aggregation with inline chunked exchange + hext2 build
            with tc.tile_pool(name="bc1", bufs=3) as bc1t, \
                 tc.tile_pool(name="gp1", bufs=4) as gp1t, \
                 tc.tile_pool(name="one1", bufs=1) as b1p1, \
                 tc.tile_pool(name="bp1", bufs=2, space="PSUM") as bp1t, \
                 tc.tile_pool(name="pa2", bufs=3) as pa2t, \
                 tc.tile_pool(name="pap2", bufs=6, space="PSUM") as pp2t:
                nc.sync.dma_start(out=hext2[0:2, :], in_=sent[:])
                nc.sync.dma_start(out=hext2[NR2 - 1:NR2, :], in_=sent[0:1, :])
                phase_c(1, bc1t, gp1t, b1p1, bp1t,
                        on_chunk_done=lambda j: phase_a2_chunk(j, pa2t, pp2t))
            with tc.tile_pool(name="bc2", bufs=4) as bc2t, \
                 tc.tile_pool(name="gp2", bufs=4) as gp2t, \
                 tc.tile_pool(name="one2", bufs=1) as b1p2, \
                 tc.tile_pool(name="bp2", bufs=2, space="PSUM") as bp2t:
                phase_c(2, bc2t, gp2t, b1p2, bp2t)

    nc.compile()
    return nc


# ----------------------------------------------------------------------------
# runner (caches compiled executable; reuses device-resident inputs)
# ----------------------------------------------------------------------------

_CACHE = {}


def _make_inputs(x, edge_index, W1, a_src1, a_dst1, b1, W2, a_src2, a_dst2, b2,
                 pre):
    import ml_dtypes
    xT = np.ascontiguousarray(x[pre["xposinv"]].T.astype(ml_dtypes.bfloat16))
    r1 = np.zeros((F_IN, RHSW), np.float32)
    r1[:, 0:HID] = W1
    r1[:, HID] = W1 @ a_src1
    r2 = np.zeros((HID, RHSW), np.float32)
    r2[:, 0:HID] = W2
    r2[:, HID] = W2 @ a_src2
    r2[:, HID + 1] = W2 @ a_dst2
    w1d = (W1 @ a_dst1).astype(ml_dtypes.bfloat16)[:, None]
    b1rep = np.tile(b1[None, :], (P, 1)).astype(np.float32)
    b2rep = np.tile(b2[None, :], (P, 1)).astype(np.float32)
    core_of, pos = pre["core_of"], pre["pos"]
    xs_all = np.zeros((NCORES, F_IN, S), np.float32)
    xs_all[core_of, :, pos] = x          # [N,F] rows scattered
    xs_all = np.ascontiguousarray(xs_all).astype(ml_dtypes.bfloat16)
    in_maps = []
    for c in range(NCORES):
        in_maps.append(dict(
            xT=xT, xS=np.ascontiguousarray(xs_all[c]), rhs1=r1, rhs2=r2,
            wad1=w1d, b1r=b1rep, b2r=b2rep,
            idx1=np.ascontiguousarray(pre["idx1"][c]),
            idx2=np.ascontiguousarray(pre["idx2"][c]),
            didx2=np.ascontiguousarray(pre["didx2"][c]),
            mh2=np.ascontiguousarray(pre["mh2"][c]),
        ))
    return in_maps


def _get_runner(pre):
    if "runner" in _CACHE:
        return _CACHE["runner"]

    import jax
    import numpy as _np
    from jax.sharding import Mesh, PartitionSpec
    from jax.experimental.shard_map import shard_map
    import concourse.mybir as mybir
    from concourse import bass2jax
    from concourse.bass2jax import _bass_exec_p, install_neuronx_cc_hook

    nc = _CACHE["nc"]
    install_neuronx_cc_hook()

    partition_name = (nc.partition_id_tensor.name
                      if nc.partition_id_tensor else None)
    in_names, out_names, out_avals, zero_shapes = [], [], [], []
    for alloc in nc.m.functions[0].allocations:
        if not isinstance(alloc, mybir.MemoryLocationSet):
            continue
        name = alloc.memorylocations[0].name
        if alloc.kind == "ExternalInput":
            if name != partition_name:
                in_names.append(name)
        elif alloc.kind == "ExternalOutput":
            out_names.append(name)
            shape = tuple(alloc.tensor_shape)
            dtype = mybir.dt.np(alloc.dtype)
            out_avals.append(jax.core.ShapedArray(shape, dtype))
            zero_shapes.append((shape, dtype))
    n_params = len(in_names)
    all_names = in_names + out_names
    if partition_name is not None:
        all_names.append(partition_name)

    import jax.numpy as jnp

    def _body(*args):
        operands = list(args)
        if partition_name is not None:
            operands.append(bass2jax.partition_id_tensor())
        return tuple(_bass_exec_p.bind(
            *operands, out_avals=tuple(out_avals), in_names=tuple(all_names),
            out_names=tuple(out_names), lowering_input_output_aliases=(),
            sim_require_finite=True, sim_require_nnan=True, nc=nc))

    devices = jax.devices()[:NCORES]
    mesh = Mesh(_np.asarray(devices), ("core",))
    n_outs = len(out_names)
    in_specs = (PartitionSpec("core"),) * (n_params + n_outs)
    out_specs = (PartitionSpec("core"),) * n_outs
    donate = tuple(range(n_params, n_params + n_outs))
    sharded = jax.jit(
        shard_map(_body, mesh=mesh, in_specs=in_specs, out_specs=out_specs,
                  check_rep=False),
        donate_argnums=donate, keep_unused=True)

    from jax.sharding import NamedSharding
    zsharding = NamedSharding(mesh, PartitionSpec("core"))
    zeros_fn = jax.jit(
        lambda: tuple(jnp.zeros((NCORES * sh[0], *sh[1:]), dt)
                      for sh, dt in zero_shapes),
        out_shardings=(zsharding,) * n_outs)

    def run(in_maps, n_timed=0):
        concat_in = [
            _np.concatenate([_np.asarray(in_maps[c][nm]) for c in range(NCORES)],
                            axis=0)
            for nm in in_names]
        shardings = [NamedSharding(mesh, PartitionSpec("core"))] * n_params
        dev_in = [jax.device_put(a, s) for a, s in zip(concat_in, shardings)]

        outs = sharded(*dev_in, *zeros_fn())
        for o in outs:
            o.block_until_ready()
        times = []
        if n_timed:
            import time as _t
            for _ in range(n_timed):
                z = zeros_fn()
                for zz in z:
                    zz.block_until_ready()
                t0 = _t.perf_counter()
                outs2 = sharded(*dev_in, *z)
                for o in outs2:
                    o.block_until_ready()
                times.append(_t.perf_counter() - t0)
        result = {}
        for i, nm in enumerate(out_names):
            arr = _np.asarray(outs[i]).reshape(NCORES, *out_avals[i].shape)
            result[nm] = arr
        return result, times

    _CACHE["runner"] = run
    return run


def _prepare(edge_index):
    if "pre" not in _CACHE:
        pre = _preprocess(np.asarray(edge_index))
        _CACHE["pre"] = pre
        import sys
        print(f"[kernel] pad ratios: L1 {pre['stats']['pad1']:.3f} "
              f"L2 {pre['stats']['pad2']:.3f}", file=sys.stderr)
    import os
    if "nc" not in _CACHE and not os.environ.get("GAT_NO_BUILD"):
        _CACHE["nc"] = _build_nc(_CACHE["pre"])
    return _CACHE["pre"]


def kernel(x, edge_index, W1, a_src1, a_dst1, b1, W2, a_src2, a_dst2, b2,
           n_timed=0):
    pre = _prepare(edge_index)
    in_maps = _make_inputs(np.asarray(x), np.asarray(edge_index),
                           np.asarray(W1), np.asarray(a_src1),
                           np.asarray(a_dst1), np.asarray(b1),
                           np.asarray(W2), np.asarray(a_src2),
                           np.asarray(a_dst2), np.asarray(b2), pre)
    run = _get_runner(pre)
    result, times = run(in_maps, n_timed=n_timed)
    slices = result["out"]                      # [NCORES, S, HID]
    out = np.empty((N, HID), np.float32)
    core_of, pos2 = pre["core_of"], pre["pos2"]
    # device stored row r = p*T + t for slot t*128+p
    sl = np.arange(S)
    rowmap = (sl % P) * T + sl // P
    out[np.arange(N)] = slices[core_of, rowmap[pos2]]
    if n_timed:
        kernel.last_times = times
    return out


kernel.last_times = []


# revision 45
# speedup vs baseline: 1.0180x; 1.0180x over previous
"""Bass/Trainium2 kernel for 2-layer GAT (nn_GAT_58128087384143).

Strategy (8 NeuronCores, one SPMD NEFF):
  - Destination nodes are partitioned across the 8 cores, degree-sorted and
    assigned round-robin by rank so every core's tile t holds similarly
    sized ELL rows; one shared (core, slot) layout serves both layers'
    segment softmax / aggregation.
  - Every core computes the full "hext" node table (replicated):
    hext[row(n)] = [h(n) as bf16 | al_src(n)] where h = x @ W and
    al_src = x @ (W a_src) come out of one PE matmul per 128-node tile
    (phase A), stored p-major so writes are a few large descriptors.
  - al_dst is never gathered: each core computes it for its own dst slots
    with tiny PE matmuls against xS (host-permuted x, layer 1) or its own
    relu(out1).T slice (layer 2).
  - Per layer, per dst-tile group (phase C): dma_gather fetches the 256-byte
    hext rows of all in-edge sources (degree-bucketed ELL, padded slots point
    at sentinel rows whose al = -1e30 so exp() kills them), ACT computes
    leaky_relu logits (Lrelu) + exp with a fused row-sum (no max-subtraction:
    logits are provably < 15 for this model family), DVE does the broadcast
    multiply + k-reduction, ACT applies 1/denom, DVE adds bias.
  - dma_gather indices are int16, so each hext table is addressed through
    two 32768-row windows; edges whose source row lands in the overlap go to
    whichever side minimizes the per-tile ELL widths.
  - Between layers, relu(out1).T is AllGathered in column chunks so the
    collective overlaps the tail of phase C1 and hext2 construction (phase
    A2) proceeds per-chunk as data arrives.

kernel(**inputs) -> np.ndarray [50000, 64] float32.
"""

import numpy as np

P = 128
NCORES = 8
N = 50000
F_IN = 128
HID = 64
T = 49                 # dst tiles per core
S = T * P              # 6272 dst slots per core
CONCAT = NCORES * S    # 50176
NR1 = N + 3            # rows: 0=sent_neg, 1=sent_zero, 2..N+1 nodes, N+2=sent_neg_hi
NR2 = CONCAT + 3
WIN = 32768            # int16 gather window
HIB1 = NR1 - WIN       # hi window base row
HIB2 = NR2 - WIN
ROWW = 64              # fp32 elements per hext row (256 B); h is bf16
ALS = 32               # hext f32 col of al_src
ALD = 33               # hext f32 col of al_dst (written/used by layer 2 only)
NEGINF = -1.0e30
NEG_SLOPE = 0.2
RHSW = 128             # phase-A rhs width: [W | wa_src | wa_dst | 0...]
CHUNK = 4096           # phase-A input streaming chunk (cols)
STAGE_TILES = 32       # node-tiles per hext store
KCAP = 56              # max summed ELL width per gather group
CSPLIT = (21, 20, 8)   # collective chunks, in dst tiles (small tail)
FP8_EXCHANGE = True    # ship relu(out1) as fp8 e4m3 instead of bf16


# ----------------------------------------------------------------------------
# host-side graph preprocessing
# ----------------------------------------------------------------------------

def _cumcount(keys_sorted):
    n = keys_sorted.shape[0]
    if n == 0:
        return np.zeros(0, np.int64)
    first = np.ones(n, bool)
    first[1:] = keys_sorted[1:] != keys_sorted[:-1]
    idx = np.arange(n)
    start = np.maximum.accumulate(np.where(first, idx, 0))
    return idx - start


def _pack16(flat):
    """[n] int -> [128, n//16] int16: idx j at partition j%16, col j//16,
    replicated 8x down the partitions (one copy per Q7 core pair)."""
    n = flat.shape[0]
    assert n % 16 == 0
    block = flat.reshape(n // 16, 16).T.astype(np.int16)
    return np.tile(block, (8, 1))


def _window_k(mustlo, musthi, deg):
    """Per-tile optimal ELL widths given per-slot must-lo/must-hi/total
    degrees shaped [NCORES, T, P]. Returns K_lo[T], K_hi[T]."""
    A = mustlo.max(axis=(0, 2))
    B = musthi.max(axis=(0, 2))
    D = deg.max(axis=(0, 2))
    K_lo = np.maximum(A, 1)   # >=1 so dead slots have a sentinel column
    K_hi = np.maximum(B, D - K_lo)
    K_hi = np.maximum(K_hi, 0)
    return K_lo.astype(np.int64), K_hi.astype(np.int64)


def _order_score(key_cols, mustlo_d, musthi_d, deg_d):
    order = np.lexsort(key_cols)
    ml = np.zeros(CONCAT, np.int64)
    mh = np.zeros(CONCAT, np.int64)
    dg = np.zeros(CONCAT, np.int64)
    ml[:N] = mustlo_d[order]
    mh[:N] = musthi_d[order]
    dg[:N] = deg_d[order]
    ml = ml.reshape(S, NCORES).T.reshape(NCORES, T, P)
    mh = mh.reshape(S, NCORES).T.reshape(NCORES, T, P)
    dg = dg.reshape(S, NCORES).T.reshape(NCORES, T, P)
    K_lo, K_hi = _window_k(ml, mh, dg)
    return (K_lo + K_hi).sum() * P * NCORES, order


def _side_assign(dst, mustlo_e, musthi_e, K_lo_of_dst, K_hi_of_dst,
                 deg_d, mustlo_d):
    """Choose lo/hi side per edge (flex edges fill lo up to what K_hi can't
    absorb)."""
    flex_e = ~(mustlo_e | musthi_e)
    lo_cap = K_lo_of_dst - mustlo_d
    need_lo = deg_d - mustlo_d - K_hi_of_dst
    x_d = np.clip(need_lo, 0, np.maximum(lo_cap, 0))
    order = np.lexsort((~flex_e, dst))
    pos = _cumcount(dst[order])
    flexrank = np.full(dst.shape[0], 1 << 30, np.int64)
    flexrank[order] = np.where(flex_e[order], pos, 1 << 30)
    lo_e = mustlo_e | (flexrank < x_d[dst])
    return lo_e


def _make_groups(K_lo, K_hi, cap, csplit):
    """Greedy grouping of dst tiles into gather groups with summed width
    <= cap, never straddling collective-chunk boundaries (layer 1 uses the
    boundaries so chunk j's tiles finish before its AllGather; layer 2 just
    reuses the same grouping code with one big chunk)."""
    bounds = []
    t0 = 0
    for c in csplit:
        bounds.append((t0, t0 + c))
        t0 += c
    groups = []
    for (b0, b1) in bounds:
        t = b0
        while t < b1:
            kp = int(K_lo[t] + K_hi[t])
            ts = [t]
            t += 1
            while t < b1 and kp + int(K_lo[t] + K_hi[t]) <= cap:
                kp += int(K_lo[t] + K_hi[t])
                ts.append(t)
                t += 1
            groups.append(ts)
    return groups


def _build_ell(dst, row_of_edge, lo_e, core_of_dst, pos_of_dst,
               K_lo, K_hi, hib, sent_hi_val, dead, groups):
    """Build per-core packed int16 index arrays for the per-group gathers.

    Group g covers tiles ts: one lo index block (tiles concatenated k-major)
    then one hi block, so each group needs two dma_gather calls."""
    core_e = core_of_dst[dst]
    pos_e = pos_of_dst[dst]
    side_e = (~lo_e).astype(np.int64)
    order = np.lexsort((side_e, pos_e, core_e))
    key = ((core_e[order] * S + pos_e[order]) << 1) | side_e[order]
    cc = _cumcount(key)

    KLM = int(K_lo.max())
    KHM = int(max(1, K_hi.max()))
    ell_lo = np.zeros((NCORES, S, KLM), np.int64)            # sent_neg = row 0
    ell_hi = np.full((NCORES, S, KHM), sent_hi_val, np.int64)
    oe = order
    lo_sel = lo_e[oe]
    ell_lo[core_e[oe][lo_sel], pos_e[oe][lo_sel], cc[lo_sel]] = \
        row_of_edge[oe][lo_sel]
    hi_sel = ~lo_sel
    ell_hi[core_e[oe][hi_sel], pos_e[oe][hi_sel], cc[hi_sel]] = \
        row_of_edge[oe][hi_sel] - hib
    # dead slots: first lo column -> sent_zero (row 1) so denom = 1, out = 0
    dc, dp = np.nonzero(dead)
    ell_lo[dc, dp, 0] = 1

    packs = [[] for _ in range(NCORES)]
    ginfo = []   # (idx_lo_off, n_lo, idx_hi_off, n_hi, kp) per group
    tinfo = []   # (group, lo_off, kl, hi_off, kh) per tile, tile-indexed
    col = 0
    tmap = {}
    for gi, ts in enumerate(groups):
        kls = [int(K_lo[t]) for t in ts]
        khs = [int(K_hi[t]) for t in ts]
        n_lo, n_hi = sum(kls), sum(khs)
        kp = n_lo + n_hi
        lo_off = col
        col += 8 * n_lo
        hi_off = col
        col += 8 * n_hi
        ginfo.append((lo_off, n_lo, hi_off, n_hi, kp))
        run = 0
        for i, t in enumerate(ts):
            tmap[t] = (gi, run, kls[i], n_lo + sum(khs[:i]), khs[i])
            run += kls[i]
        for c in range(NCORES):
            blks = [ell_lo[c, t * P:(t + 1) * P, :int(K_lo[t])].T.reshape(-1)
                    for t in ts]
            packs[c].append(_pack16(np.concatenate(blks)))
            if n_hi:
                blks = [ell_hi[c, t * P:(t + 1) * P, :int(K_hi[t])].T.reshape(-1)
                        for t in ts]
                packs[c].append(_pack16(np.concatenate(blks)))
    tinfo = [tmap[t] for t in range(T)]
    idx = np.stack([np.concatenate(p, axis=1) for p in packs])  # [NC,128,C]
    return np.ascontiguousarray(idx), (ginfo, tinfo, groups), col


def _rowmap_pmajor(total):
    """DRAM row offset for each sequential stream position, matching phase
    A1's p-major stage stores over CHUNK-column spans."""
    rm = np.empty(total, np.int64)
    for base in range(0, total, CHUNK):
        cols = min(CHUNK, total - base)
        idx = np.arange(cols)
        if cols % P == 0:
            ntile = cols // P
            rm[base:base + cols] = base + (idx % P) * ntile + idx // P
        else:
            rm[base:base + cols] = base + idx
    return rm


def _rowmap_csplit():
    """Per-core row offset of each slot for phase A2's per-collective-chunk
    p-major stores."""
    rm = np.empty(S, np.int64)
    base = 0
    for c in CSPLIT:
        cols = c * P
        idx = np.arange(cols)
        rm[base:base + cols] = base + (idx % P) * c + idx // P
        base += cols
    return rm


def _preprocess(edge_index):
    src = np.concatenate([edge_index[0].astype(np.int64), np.arange(N)])
    dst = np.concatenate([edge_index[1].astype(np.int64), np.arange(N)])
    deg_d = np.bincount(dst, minlength=N)
    outdeg = np.bincount(src, minlength=N)

    # ---------- layer-1 row placement (via host x permutation) ----------
    # top-out-degree nodes go to rows in the two-window overlap so the most
    # edges become side-flexible; the rest alternate lo/hi by out-degree.
    rows_of_pos = _rowmap_pmajor(N) + 2
    ov = (rows_of_pos >= HIB1) & (rows_of_pos < WIN)
    over_pos = np.where(ov)[0]
    lo_pos = np.where(rows_of_pos < HIB1)[0]
    hi_pos = np.where(rows_of_pos >= WIN)[0]
    by_out = np.argsort(-outdeg, kind="stable")
    pos_of_node = np.empty(N, np.int64)
    pos_of_node[by_out[:over_pos.size]] = over_pos
    rest = by_out[over_pos.size:]
    nlo, nhi = lo_pos.size, hi_pos.size
    sel_lo, sel_hi = [], []
    li = hi = 0
    for i, n in enumerate(rest):
        if (i % 2 == 0 and li < nlo) or hi >= nhi:
            sel_lo.append(n); li += 1
        else:
            sel_hi.append(n); hi += 1
    pos_of_node[np.array(sel_lo)] = lo_pos
    pos_of_node[np.array(sel_hi)] = hi_pos
    xposinv = np.empty(N, np.int64)
    xposinv[pos_of_node] = np.arange(N)      # stream position -> node
    rowmap1 = rows_of_pos[pos_of_node]       # hext1 row of node
    row1 = rowmap1[src]
    mustlo1_e = row1 < HIB1
    musthi1_e = row1 >= WIN
    mustlo1_d = np.bincount(dst[mustlo1_e], minlength=N)
    musthi1_d = np.bincount(dst[musthi1_e], minlength=N)

    blk_map = _rowmap_csplit()
    cands = [
        (-deg_d, -mustlo1_d),
        (-musthi1_d, -mustlo1_d),
        (-mustlo1_d, -musthi1_d),
        (-(mustlo1_d + musthi1_d), -deg_d),
        (-deg_d, -(mustlo1_d - musthi1_d)),
        (-np.maximum(mustlo1_d, musthi1_d), -deg_d),
    ]
    best = None
    for kc in cands:
        score, order = _order_score(kc, mustlo1_d, musthi1_d, deg_d)
        if best is None or score < best[0]:
            best = (score, order)
    slots1, order1 = best
    rank1 = np.empty(N, np.int64)
    rank1[order1] = np.arange(N)
    core_of = rank1 % NCORES
    pos = rank1 // NCORES
    dead = np.ones((NCORES, S), np.uint8)
    dead[core_of, pos] = 0

    ml = np.zeros(CONCAT, np.int64); mh = np.zeros(CONCAT, np.int64)
    dg = np.zeros(CONCAT, np.int64)
    ml[:N] = mustlo1_d[order1]; mh[:N] = musthi1_d[order1]
    dg[:N] = deg_d[order1]
    K1_lo, K1_hi = _window_k(ml.reshape(S, NCORES).T.reshape(NCORES, T, P),
                             mh.reshape(S, NCORES).T.reshape(NCORES, T, P),
                             dg.reshape(S, NCORES).T.reshape(NCORES, T, P))
    lo1_e = _side_assign(dst, mustlo1_e, musthi1_e, K1_lo[pos // P],
                         K1_hi[pos // P], deg_d, mustlo1_d)
    groups1 = _make_groups(K1_lo, K1_hi, KCAP, CSPLIT)
    idx1, offs1, C1 = _build_ell(dst, row1, lo1_e, core_of, pos,
                                 K1_lo, K1_hi, HIB1, NR1 - 1 - HIB1, dead,
                                 groups1)

    # ---------- layer 2 (own dst ordering; source rows are concat slots) --
    crow = core_of * S + pos
    rowmap2cat = ((crow // S) * S + blk_map[crow % S]) + 2
    r2 = rowmap2cat[src]
    mustlo2_e = r2 < HIB2
    musthi2_e = r2 >= WIN
    mustlo2_d = np.bincount(dst[mustlo2_e], minlength=N)
    musthi2_d = np.bincount(dst[musthi2_e], minlength=N)

    cands2 = [
        (-deg_d, -mustlo2_d, core_of),
        (-musthi2_d, -mustlo2_d, core_of),
        (-mustlo2_d, -musthi2_d, core_of),
        (-deg_d, -(mustlo2_d - musthi2_d), core_of),
        (-(mustlo2_d + musthi2_d), -deg_d, core_of),
        (-np.maximum(mustlo2_d, musthi2_d), -deg_d, core_of),
    ]
    best2 = None
    for kc in cands2:
        o2 = np.lexsort(kc)
        p2 = np.empty(N, np.int64)
        p2[o2] = _cumcount(core_of[o2])
        ml = np.zeros((NCORES, S), np.int64)
        mh = np.zeros((NCORES, S), np.int64)
        dg2 = np.zeros((NCORES, S), np.int64)
        ml[core_of, p2] = mustlo2_d
        mh[core_of, p2] = musthi2_d
        dg2[core_of, p2] = deg_d
        klo, khi = _window_k(ml.reshape(NCORES, T, P),
                             mh.reshape(NCORES, T, P),
                             dg2.reshape(NCORES, T, P))
        score = int((klo + khi).sum())
        if best2 is None or score < best2[0]:
            best2 = (score, p2, klo, khi)
    _, pos2, K2_lo, K2_hi = best2
    slots2 = int((K2_lo + K2_hi).sum()) * P * NCORES
    dead2 = np.ones((NCORES, S), np.uint8)
    dead2[core_of, pos2] = 0

    lo2_e = _side_assign(dst, mustlo2_e, musthi2_e, K2_lo[pos2 // P],
                         K2_hi[pos2 // P], deg_d, mustlo2_d)
    groups2 = _make_groups(K2_lo, K2_hi, KCAP, (T,))
    idx2, offs2, C2 = _build_ell(dst, r2, lo2_e, core_of, pos2,
                                 K2_lo, K2_hi, HIB2, NR2 - 1 - HIB2, dead2,
                                 groups2)

    # ---------- per-dst-row al_dst2 gather (layer 2 phase B) ----------
    rect_lo = np.zeros((NCORES, S), np.int64)
    rect_hi = np.full((NCORES, S), NR2 - 1 - HIB2, np.int64)
    mask_hi = np.zeros((NCORES, S), np.uint8)
    r = rowmap2cat
    is_lo = r < WIN
    rect_lo[core_of[is_lo], pos2[is_lo]] = r[is_lo]
    ih = ~is_lo
    rect_hi[core_of[ih], pos2[ih]] = r[ih] - HIB2
    mask_hi[core_of[ih], pos2[ih]] = 1
    # dead slots: lo sentinel-zero row so al_dst = 0
    rect_lo[dead2 > 0] = 1
    didx2 = np.stack([np.concatenate(
        [_pack16(rect_lo[c]), _pack16(rect_hi[c])], axis=1)
        for c in range(NCORES)])
    mh2 = np.ascontiguousarray(np.stack(
        [mask_hi[c].reshape(T, P).T for c in range(NCORES)]))

    stats = dict(slots1=int(slots1), slots2=int(slots2),
                 edges=int(dst.shape[0]),
                 pad1=float(slots1) / dst.shape[0],
                 pad2=float(slots2) / dst.shape[0])
    return dict(idx1=idx1, offs1=offs1, C1=C1, K1_lo=K1_lo, K1_hi=K1_hi,
                rowmap1=rowmap1, rowmap2cat=rowmap2cat, crow=crow,
                xposinv=xposinv,
                idx2=idx2, offs2=offs2, C2=C2, K2_lo=K2_lo, K2_hi=K2_hi,
                didx2=didx2, mh2=mh2,
                core_of=core_of, pos=pos, pos2=pos2, blk_map=blk_map,
                stats=stats)


# ----------------------------------------------------------------------------
# device kernel
# ----------------------------------------------------------------------------

def _build_nc(pre):
    import concourse.bass as bass
    import concourse.mybir as mybir
    import concourse.tile as tile
    from concourse import bacc
    from concourse.masks import make_identity

    f32 = mybir.dt.float32
    bf16 = mybir.dt.bfloat16
    i16 = mybir.dt.int16
    x8 = mybir.dt.float8e4 if FP8_EXCHANGE else bf16
    AF = mybir.ActivationFunctionType
    OP = mybir.AluOpType
    AX = mybir.AxisListType

    offs1, offs2 = pre["offs1"], pre["offs2"]
    C1, C2 = pre["C1"], pre["C2"]

    nc = bacc.Bacc("TRN2", num_devices=NCORES, target_bir_lowering=False)

    xT = nc.dram_tensor("xT", [F_IN, N], bf16, kind="ExternalInput")
    xS = nc.dram_tensor("xS", [F_IN, S], bf16, kind="ExternalInput")
    rhs1 = nc.dram_tensor("rhs1", [F_IN, RHSW], f32, kind="ExternalInput")
    rhs2 = nc.dram_tensor("rhs2", [HID, RHSW], f32, kind="ExternalInput")
    wad1 = nc.dram_tensor("wad1", [F_IN, 1], bf16, kind="ExternalInput")
    b1r = nc.dram_tensor("b1r", [P, HID], f32, kind="ExternalInput")
    b2r = nc.dram_tensor("b2r", [P, HID], f32, kind="ExternalInput")
    idx1 = nc.dram_tensor("idx1", [P, C1], i16, kind="ExternalInput")
    idx2 = nc.dram_tensor("idx2", [P, C2], i16, kind="ExternalInput")
    didx2 = nc.dram_tensor("didx2", [P, 2 * (S // 16)], i16,
                           kind="ExternalInput")
    mh2 = nc.dram_tensor("mh2", [P, T], mybir.dt.uint8, kind="ExternalInput")
    out2 = nc.dram_tensor("out", [S, HID], f32, kind="ExternalOutput")

    hext1 = nc.dram_tensor("hext1", [NR1, ROWW], f32, kind="Internal")
    hext2 = nc.dram_tensor("hext2", [NR2, ROWW], f32, kind="Internal")
    o1c = [nc.dram_tensor(f"o1c{j}", [HID, CSPLIT[j] * P], x8, kind="Internal")
           for j in range(len(CSPLIT))]
    ag = [nc.dram_tensor(f"ag{j}", [NCORES, HID, CSPLIT[j] * P], x8,
                         kind="Internal", addr_space="Shared")
          for j in range(len(CSPLIT))]

    KMAX = int(max(max(g[4] for g in offs1[0]), max(g[4] for g in offs2[0])))
    NTGMAX = max(max(len(ts) for ts in offs1[2]),
                 max(len(ts) for ts in offs2[2]))

    with tile.TileContext(nc) as tc:
        with tc.tile_pool(name="const", bufs=1) as cp:
            rhs1_sb = cp.tile([F_IN, RHSW], bf16)
            nc.gpsimd.dma_start(out=rhs1_sb[:], in_=rhs1[:, :])
            rhs2_sb = cp.tile([HID, RHSW], bf16)
            nc.gpsimd.dma_start(out=rhs2_sb[:], in_=rhs2[:, :])
            wad1_sb = cp.tile([F_IN, 1], bf16)
            nc.gpsimd.dma_start(out=wad1_sb[:], in_=wad1[:, :])
            b1_sb = cp.tile([P, HID], f32)
            nc.sync.dma_start(out=b1_sb[:], in_=b1r[:, :])
            b2_sb = cp.tile([P, HID], f32)
            nc.sync.dma_start(out=b2_sb[:], in_=b2r[:, :])
            ident = cp.tile([P, P], f32)
            make_identity(nc, ident[:])
            xS_sb = cp.tile([F_IN, S], bf16)
            nc.sync.dma_start(out=xS_sb[:], in_=xS[:, :])
            o1T_sb = cp.tile([HID, S], bf16)
            ald1 = cp.tile([P, T], f32)
            ald1_02 = cp.tile([P, T], f32)
            # sentinel rows: row0 al=-1e30 (pad), row1 al=0 (dead slots)
            sent = cp.tile([2, ROWW], f32)
            nc.vector.memset(sent[:], 0.0)
            nc.vector.memset(sent[0:1, ALS:ALS + 1], NEGINF)
            # index tables: load up front so they never queue behind
            # collective-gated DMAs
            idx1_sb = cp.tile([P, C1], i16)
            nc.sync.dma_start(out=idx1_sb[:], in_=idx1[:, :])
            idx2_sb = cp.tile([P, C2], i16)
            nc.sync.dma_start(out=idx2_sb[:], in_=idx2[:, :])
            didx2_sb = cp.tile([P, 2 * (S // 16)], i16)
            nc.sync.dma_start(out=didx2_sb[:], in_=didx2[:, :])
            mh2_sb = cp.tile([P, T], mybir.dt.uint8)
            nc.sync.dma_start(out=mh2_sb[:], in_=mh2[:, :])

            def phase_b1():
                """al_dst1 for this core's own dst slots via tiny matmuls
                against the host-permuted x (no gather needed)."""
                with tc.tile_pool(name="pb1", bufs=1, space="PSUM") as pb:
                    ps = pb.tile([P, T], f32, space="PSUM")
                    for t in range(T):
                        nc.tensor.matmul(out=ps[:, t:t + 1],
                                         lhsT=xS_sb[:, t * P:(t + 1) * P],
                                         rhs=wad1_sb[:],
                                         start=True, stop=True,
                                         skip_group_check=True)
                    nc.scalar.activation(out=ald1[:], in_=ps[:], func=AF.Copy)
                    nc.vector.tensor_scalar(out=ald1_02[:], in0=ald1[:],
                                            scalar1=NEG_SLOPE, scalar2=None,
                                            op0=OP.mult)

            def phase_a(layer):
                assert layer == 1
                hext = hext1
                rhs_sb = rhs1_sb
                kdim = F_IN
                with tc.tile_pool(name=f"pa{layer}", bufs=3) as pa, \
                     tc.tile_pool(name=f"pap{layer}", bufs=6, space="PSUM") as pp:
                    nc.sync.dma_start(out=hext[0:2, :], in_=sent[:])
                    nc.sync.dma_start(out=hext[NR1 - 1:NR1, :],
                                      in_=sent[0:1, :])

                    spans = [(c0, min(CHUNK, N - c0), 0, 2 + c0)
                             for c0 in range(0, N, CHUNK)]
                    for c0, cols, blk, rowbase in spans:
                        in_sb = pa.tile([kdim, CHUNK], bf16, tag="pa_in")
                        nc.sync.dma_start(out=in_sb[:, 0:cols],
                                          in_=xT[:, c0:c0 + cols])
                        ntile = (cols + P - 1) // P
                        stage = pa.tile([P, STAGE_TILES, ROWW], f32, tag="pa_st")
                        stage_bf = stage[:].bitcast(bf16)
                        QUAD = 4
                        nt = 0
                        while nt < ntile:
                            q = min(QUAD, ntile - nt)
                            rows = [min(P, cols - (nt + i) * P)
                                    for i in range(q)]
                            if rows[0] == P:
                                while q > 1 and rows[q - 1] < P:
                                    q -= 1
                            else:
                                q = 1
                            r = rows[0] if q == 1 else P
                            ps = pp.tile([P, QUAD, RHSW], f32, space="PSUM")
                            for i in range(q):
                                nc.tensor.matmul(
                                    out=ps[0:r, i, :],
                                    lhsT=in_sb[:, (nt + i) * P:
                                               (nt + i) * P + r],
                                    rhs=rhs_sb[:],
                                    start=True, stop=True,
                                    skip_group_check=True)
                            nc.scalar.activation(
                                out=stage_bf[0:r, nt:nt + q, 0:HID],
                                in_=ps[0:r, 0:q, 0:HID], func=AF.Copy)
                            nc.vector.tensor_copy(
                                out=stage[0:r, nt:nt + q, ALS:ALS + 1],
                                in_=ps[0:r, 0:q, HID:HID + 1])
                            nt += q
                        full = cols // P
                        rem = cols - full * P
                        if rem == 0:
                            nc.sync.dma_start(
                                out=hext[rowbase:rowbase + cols, :]
                                .rearrange("(p n) w -> p n w", p=P),
                                in_=stage[:, 0:full, :])
                        else:
                            if full:
                                nc.sync.dma_start(
                                    out=hext[rowbase:rowbase + full * P, :]
                                    .rearrange("(n p) w -> p n w", p=P),
                                    in_=stage[:, 0:full, :])
                            nc.sync.dma_start(
                                out=hext[rowbase + full * P:
                                         rowbase + full * P + rem, :]
                                .rearrange("(n p) w -> p n w", p=rem),
                                in_=stage[0:rem, full:full + 1, :])

            def phase_c(layer, bc, gp, b1p, bp, on_chunk_done=None):
                hext = hext1 if layer == 1 else hext2
                hib = HIB1 if layer == 1 else HIB2
                offs = offs1 if layer == 1 else offs2
                idx_t = idx1 if layer == 1 else idx2
                cdim = C1 if layer == 1 else C2
                b_sb = b1_sb if layer == 1 else b2_sb

                src_lo = hext[0:WIN, :]
                src_hi = hext[hib:hib + WIN, :]
                ginfo, tinfo, groups = offs

                idx_sb = idx1_sb if layer == 1 else idx2_sb
                if layer == 1:
                    ald = ald1
                    ald02 = ald1_02
                else:
                    o2_sb = b1p.tile([P, T, HID], f32)
                    # al_dst2 per own dst slot: gather hext2 rows, read ALD
                    Gd_lo = b1p.tile([P, T, ROWW], f32)
                    nc.gpsimd.dma_gather(
                        out_ap=Gd_lo[:], in_ap=src_lo,
                        idxs_ap=didx2_sb[:, 0:S // 16],
                        num_idxs=S, num_idxs_reg=S, elem_size=ROWW,
                        single_packet=False)
                    Gd_hi = b1p.tile([P, T, ROWW], f32)
                    nc.gpsimd.dma_gather(
                        out_ap=Gd_hi[:], in_ap=src_hi,
                        idxs_ap=didx2_sb[:, S // 16:2 * (S // 16)],
                        num_idxs=S, num_idxs_reg=S, elem_size=ROWW,
                        single_packet=False)
                    ald = b1p.tile([P, T], f32)
                    nc.vector.tensor_copy(out=ald[:], in_=Gd_lo[:, :, ALD])
                    nc.vector.copy_predicated(out=ald[:], mask=mh2_sb[:],
                                              data=Gd_hi[:, :, ALD])
                    ald02 = b1p.tile([P, T], f32)
                    nc.vector.tensor_scalar(out=ald02[:], in0=ald[:],
                                            scalar1=NEG_SLOPE, scalar2=None,
                                            op0=OP.mult)

                chunk_end = []
                t0 = 0
                for c in CSPLIT:
                    chunk_end.append(t0 + c)
                    t0 += c
                pending = []

                def emit_exchange(j):
                    nc.gpsimd.collective_compute(
                        kind="AllGather", op=OP.bypass,
                        replica_groups=[list(range(NCORES))],
                        ins=[o1c[j][:, :]], outs=[ag[j][:, :, :]])
                    # hext2 build for the PREVIOUS chunk: its collective has
                    # landed by now, so the SP queue never parks on an
                    # unfinished AllGather in front of later o1c stores.
                    if on_chunk_done is not None and j > 0:
                        on_chunk_done(j - 1)

                for gi, ts in enumerate(groups):
                    ilo, n_lo, ihi, n_hi, kp = ginfo[gi]
                    G = gp.tile([P, KMAX, ROWW], f32, tag="G")
                    nc.gpsimd.dma_gather(
                        out_ap=G[:, 0:n_lo, :], in_ap=src_lo,
                        idxs_ap=idx_sb[:, ilo:ilo + 8 * n_lo],
                        num_idxs=P * n_lo, num_idxs_reg=P * n_lo,
                        elem_size=ROWW, single_packet=False)
                    if n_hi:
                        nc.gpsimd.dma_gather(
                            out_ap=G[:, n_lo:kp, :], in_ap=src_hi,
                            idxs_ap=idx_sb[:, ihi:ihi + 8 * n_hi],
                            num_idxs=P * n_hi, num_idxs_reg=P * n_hi,
                            elem_size=ROWW, single_packet=False)
                    Gh = G[:].bitcast(bf16)
                    ntg = len(ts)
                    exg = bc.tile([P, KMAX], f32, tag="exg")
                    e0 = bc.tile([P, KMAX], f32, tag="e0")
                    e1 = bc.tile([P, KMAX], f32, tag="e1")
                    den = bc.tile([P, 2, NTGMAX], f32, tag="den")
                    rec = bc.tile([P, NTGMAX], f32, tag="rec")
                    any_hi = any(tinfo[t][4] for t in ts)
                    if any_hi:
                        nc.vector.memset(den[:, 1, 0:ntg], 0.0)
                    reds = []
                    # pass 1: ex = exp(leaky_relu(al_src + al_dst)) in group
                    # layout; per-range row-sums -> den.  No max subtraction:
                    # logits are bounded (~15) for this model family.
                    for ti, t in enumerate(ts):
                        _, lo_off, kl, hi_off, kh = tinfo[t]
                        ad = ald[:, t:t + 1]
                        ad02 = ald02[:, t:t + 1]
                        for si, (o, k) in enumerate(((lo_off, kl),
                                                     (hi_off, kh))):
                            if k == 0:
                                continue
                            # leaky_relu(x + ad) = max(x + ad, 0.2x + 0.2ad)
                            nc.scalar.activation(
                                out=e0[:, o:o + k], in_=G[:, o:o + k, ALS],
                                func=AF.Identity, bias=ad, scale=1.0)
                            nc.scalar.activation(
                                out=e1[:, o:o + k], in_=G[:, o:o + k, ALS],
                                func=AF.Identity, bias=ad02,
                                scale=NEG_SLOPE)
                            nc.vector.tensor_tensor(
                                out=e1[:, o:o + k], in0=e0[:, o:o + k],
                                in1=e1[:, o:o + k], op=OP.max)
                            nc.scalar.activation(
                                out=exg[:, o:o + k], in_=e1[:, o:o + k],
                                func=AF.Exp,
                                accum_out=den[:, si, ti:ti + 1])
                        # weighted sum can start before the denominators are
                        # merged — only the final scale needs 1/den
                        kt = kl + kh
                        prod = bc.tile([P, KMAX, HID], bf16, tag="prod")
                        for (o, k, d0) in ((lo_off, kl, 0),
                                           (hi_off, kh, kl)):
                            if k == 0:
                                continue
                            nc.vector.tensor_tensor(
                                out=prod[:, d0:d0 + k, :],
                                in0=Gh[:, o:o + k, 0:HID],
                                in1=exg[:, o:o + k, None]
                                .to_broadcast([P, k, HID]),
                                op=OP.mult)
                        red = bc.tile([P, HID], f32, tag=f"red{ti}")
                        nc.vector.tensor_reduce(
                            out=red[:], in_=prod[:, 0:kt, :].rearrange(
                                "p k f -> p f k"),
                            axis=AX.X, op=OP.add)
                        reds.append(red)
                    if any_hi:
                        nc.vector.tensor_tensor(out=den[:, 0, 0:ntg],
                                                in0=den[:, 0, 0:ntg],
                                                in1=den[:, 1, 0:ntg],
                                                op=OP.add)
                    nc.vector.reciprocal(out=rec[:, 0:ntg],
                                         in_=den[:, 0, 0:ntg])
                    # pass 2: normalize + bias + store per tile
                    for ti, t in enumerate(ts):
                        outt = bc.tile([P, HID], f32, tag="outt")
                        nc.scalar.activation(out=outt[:], in_=reds[ti][:],
                                             func=AF.Copy,
                                             scale=rec[:, ti:ti + 1])
                        if layer == 1:
                            nc.vector.tensor_tensor(out=outt[:], in0=outt[:],
                                                    in1=b_sb[:], op=OP.add)
                            psT = bp.tile([HID, P], f32, space="PSUM")
                            nc.tensor.transpose(out=psT[:], in_=outt[:],
                                                identity=ident[:])
                            nc.scalar.activation(
                                out=o1T_sb[:, t * P:(t + 1) * P],
                                in_=psT[:], func=AF.Relu)
                        else:
                            nc.vector.tensor_tensor(out=o2_sb[:, t, :],
                                                    in0=outt[:], in1=b_sb[:],
                                                    op=OP.add)
                    if layer == 1 and ts[-1] + 1 in chunk_end:
                        j = chunk_end.index(ts[-1] + 1)
                        cbase = (chunk_end[j - 1] if j else 0) * P
                        cw = CSPLIT[j] * P
                        if FP8_EXCHANGE:
                            o1x = b1p.tile([HID, S], x8, tag="o1x")
                            nc.vector.tensor_copy(
                                out=o1x[:, cbase:cbase + cw],
                                in_=o1T_sb[:, cbase:cbase + cw])
                            nc.sync.dma_start(out=o1c[j][:, :],
                                              in_=o1x[:, cbase:cbase + cw])
                        else:
                            nc.sync.dma_start(out=o1c[j][:, :],
                                              in_=o1T_sb[:, cbase:cbase + cw])
                        # the collective itself is emitted two groups later:
                        # it parks Pool SEQ until o1c lands, so give the next
                        # chunk's gather preps a head start in the queue
                        pending.append((gi + 2, j))
                    while pending and pending[0][0] <= gi:
                        emit_exchange(pending.pop(0)[1])

                while pending:
                    emit_exchange(pending.pop(0)[1])
                if layer == 1 and on_chunk_done is not None:
                    on_chunk_done(len(CSPLIT) - 1)

                if layer == 2:
                    nc.sync.dma_start(
                        out=out2[:, :].rearrange("(p t) f -> p t f", p=P),
                        in_=o2_sb[:])

            def phase_a2_chunk(j, pa, pp):
                """hext2 rows for collective chunk j, all 8 source blocks."""
                cbase = sum(CSPLIT[:j]) * P
                cols = CSPLIT[j] * P
                ntile = CSPLIT[j]
                for blk in range(NCORES):
                    rowbase = 2 + blk * S + cbase
                    in_sb = pa.tile([HID, CHUNK], bf16, tag="pa_in")
                    if FP8_EXCHANGE:
                        raw = pa.tile([HID, CHUNK], x8, tag="pa_raw")
                        nc.sync.dma_start(out=raw[:, 0:cols],
                                          in_=ag[j][blk, :, :])
                        nc.vector.tensor_copy(out=in_sb[:, 0:cols],
                                              in_=raw[:, 0:cols])
                    else:
                        nc.sync.dma_start(out=in_sb[:, 0:cols],
                                          in_=ag[j][blk, :, :])
                    stage = pa.tile([P, STAGE_TILES, ROWW], f32, tag="pa_st")
                    stage_bf = stage[:].bitcast(bf16)
                    QUAD = 4
                    nt = 0
                    while nt < ntile:
                        q = min(QUAD, ntile - nt)
                        ps = pp.tile([P, QUAD, RHSW], f32, space="PSUM")
                        for i in range(q):
                            nc.tensor.matmul(
                                out=ps[:, i, :],
                                lhsT=in_sb[:, (nt + i) * P:(nt + i + 1) * P],
                                rhs=rhs2_sb[:],
                                start=True, stop=True,
                                skip_group_check=True)
                        nc.scalar.activation(
                            out=stage_bf[:, nt:nt + q, 0:HID],
                            in_=ps[:, 0:q, 0:HID], func=AF.Copy)
                        nc.vector.tensor_copy(
                            out=stage[:, nt:nt + q, ALS:ALD + 1],
                            in_=ps[:, 0:q, HID:HID + 2])
                        nt += q
                    nc.sync.dma_start(
                        out=hext2[rowbase:rowbase + cols, :]
                        .rearrange("(p n) w -> p n w", p=P),
                        in_=stage[:, 0:ntile, :])

            phase_a(1)
            phase_b1()
            # layer-1 aggregation with inline chunked exchange + hext2 build
            with tc.tile_pool(name="bc1", bufs=3) as bc1t, \
                 tc.tile_pool(name="gp1", bufs=4) as gp1t, \
                 tc.tile_pool(name="one1", bufs=1) as b1p1, \
                 tc.tile_pool(name="bp1", bufs=2, space="PSUM") as bp1t, \
                 tc.tile_pool(name="pa2", bufs=3) as pa2t, \
                 tc.tile_pool(name="pap2", bufs=6, space="PSUM") as pp2t:
                nc.sync.dma_start(out=hext2[0:2, :], in_=sent[:])
                nc.sync.dma_start(out=hext2[NR2 - 1:NR2, :], in_=sent[0:1, :])
                phase_c(1, bc1t, gp1t, b1p1, bp1t,
                        on_chunk_done=lambda j: phase_a2_chunk(j, pa2t, pp2t))
            with tc.tile_pool(name="bc2", bufs=4) as bc2t, \
                 tc.tile_pool(name="gp2", bufs=4) as gp2t, \
                 tc.tile_pool(name="one2", bufs=1) as b1p2, \
                 tc.tile_pool(name="bp2", bufs=2, space="PSUM") as bp2t:
                phase_c(2, bc2t, gp2t, b1p2, bp2t)

    nc.compile()
    return nc


# ----------------------------------------------------------------------------
# runner (caches compiled executable; reuses device-resident inputs)
# ----------------------------------------------------------------------------

_CACHE = {}


def _make_inputs(x, edge_index, W1, a_src1, a_dst1, b1, W2, a_src2, a_dst2, b2,
                 pre):
    import ml_dtypes
    xT = np.ascontiguousarray(x[pre["xposinv"]].T.astype(ml_dtypes.bfloat16))
    r1 = np.zeros((F_IN, RHSW), np.float32)
    r1[:, 0:HID] = W1
    r1[:, HID] = W1 @ a_src1
    r2 = np.zeros((HID, RHSW), np.float32)
    r2[:, 0:HID] = W2
    r2[:, HID] = W2 @ a_src2
    r2[:, HID + 1] = W2 @ a_dst2
    w1d = (W1 @ a_dst1).astype(ml_dtypes.bfloat16)[:, None]
    b1rep = np.tile(b1[None, :], (P, 1)).astype(np.float32)
    b2rep = np.tile(b2[None, :], (P, 1)).astype(np.float32)
    core_of, pos = pre["core_of"], pre["pos"]
    xs_all = np.zeros((NCORES, F_IN, S), np.float32)
    xs_all[core_of, :, pos] = x          # [N,F] rows scattered
    xs_all = np.ascontiguousarray(xs_all).astype(ml_dtypes.bfloat16)
    in_maps = []
    for c in range(NCORES):
        in_maps.append(dict(
            xT=xT, xS=np.ascontiguousarray(xs_all[c]), rhs1=r1, rhs2=r2,
            wad1=w1d, b1r=b1rep, b2r=b2rep,
            idx1=np.ascontiguousarray(pre["idx1"][c]),
            idx2=np.ascontiguousarray(pre["idx2"][c]),
            didx2=np.ascontiguousarray(pre["didx2"][c]),
            mh2=np.ascontiguousarray(pre["mh2"][c]),
        ))
    return in_maps


def _get_runner(pre):
    if "runner" in _CACHE:
        return _CACHE["runner"]

    import jax
    import numpy as _np
    from jax.sharding import Mesh, PartitionSpec
    from jax.experimental.shard_map import shard_map
    import concourse.mybir as mybir
    from concourse import bass2jax
    from concourse.bass2jax import _bass_exec_p, install_neuronx_cc_hook

    nc = _CACHE["nc"]
    install_neuronx_cc_hook()

    partition_name = (nc.partition_id_tensor.name
                      if nc.partition_id_tensor else None)
    in_names, out_names, out_avals, zero_shapes = [], [], [], []
    for alloc in nc.m.functions[0].allocations:
        if not isinstance(alloc, mybir.MemoryLocationSet):
            continue
        name = alloc.memorylocations[0].name
        if alloc.kind == "ExternalInput":
            if name != partition_name:
                in_names.append(name)
        elif alloc.kind == "ExternalOutput":
            out_names.append(name)
            shape = tuple(alloc.tensor_shape)
            dtype = mybir.dt.np(alloc.dtype)
            out_avals.append(jax.core.ShapedArray(shape, dtype))
            zero_shapes.append((shape, dtype))
    n_params = len(in_names)
    all_names = in_names + out_names
    if partition_name is not None:
        all_names.append(partition_name)

    import jax.numpy as jnp

    def _body(*args):
        operands = list(args)
        if partition_name is not None:
            operands.append(bass2jax.partition_id_tensor())
        return tuple(_bass_exec_p.bind(
            *operands, out_avals=tuple(out_avals), in_names=tuple(all_names),
            out_names=tuple(out_names), lowering_input_output_aliases=(),
            sim_require_finite=True, sim_require_nnan=True, nc=nc))

    devices = jax.devices()[:NCORES]
    mesh = Mesh(_np.asarray(devices), ("core",))
    n_outs = len(out_names)
    in_specs = (PartitionSpec("core"),) * (n_params + n_outs)
    out_specs = (PartitionSpec("core"),) * n_outs
    donate = tuple(range(n_params, n_params + n_outs))
    sharded = jax.jit(
        shard_map(_body, mesh=mesh, in_specs=in_specs, out_specs=out_specs,
                  check_rep=False),
        donate_argnums=donate, keep_unused=True)

    from jax.sharding import NamedSharding
    zsharding = NamedSharding(mesh, PartitionSpec("core"))
    zeros_fn = jax.jit(
        lambda: tuple(jnp.zeros((NCORES * sh[0], *sh[1:]), dt)
                      for sh, dt in zero_shapes),
        out_shardings=(zsharding,) * n_outs)

    def run(in_maps, n_timed=0):
        concat_in = [
            _np.concatenate([_np.asarray(in_maps[c][nm]) for c in range(NCORES)],
                            axis=0)
            for nm in in_names]
        shardings = [NamedSharding(mesh, PartitionSpec("core"))] * n_params
        dev_in = [jax.device_put(a, s) for a, s in zip(concat_in, shardings)]

        outs = sharded(*dev_in, *zeros_fn())
        for o in outs:
            o.block_until_ready()
        times = []
        if n_timed:
            import time as _t
            for _ in range(n_timed):
                z = zeros_fn()
                for zz in z:
                    zz.block_until_ready()
                t0 = _t.perf_counter()
                outs2 = sharded(*dev_in, *z)
                for o in outs2:
                    o.block_until_ready()
                times.append(_t.perf_counter() - t0)
        result = {}
        for i, nm in enumerate(out_names):
            arr = _np.asarray(outs[i]).reshape(NCORES, *out_avals[i].shape)
            result[nm] = arr
        return result, times

    _CACHE["runner"] = run
    return run


def _prepare(edge_index):
    if "pre" not in _CACHE:
        pre = _preprocess(np.asarray(edge_index))
        _CACHE["pre"] = pre
        import sys
        print(f"[kernel] pad ratios: L1 {pre['stats']['pad1']:.3f} "
              f"L2 {pre['stats']['pad2']:.3f}", file=sys.stderr)
    import os
    if "nc" not in _CACHE and not os.environ.get("GAT_NO_BUILD"):
        _CACHE["nc"] = _build_nc(_CACHE["pre"])
    return _CACHE["pre"]


def kernel(x, edge_index, W1, a_src1, a_dst1, b1, W2, a_src2, a_dst2, b2,
           n_timed=0):
    pre = _prepare(edge_index)
    in_maps = _make_inputs(np.asarray(x), np.asarray(edge_index),
                           np.asarray(W1), np.asarray(a_src1),
                           np.asarray(a_dst1), np.asarray(b1),
                           np.asarray(W2), np.asarray(a_src2),
                           np.asarray(a_dst2), np.asarray(b2), pre)
    run = _get_runner(pre)
    result, times = run(in_maps, n_timed=n_timed)
    slices = result["out"]                      # [NCORES, S, HID]
    out = np.empty((N, HID), np.float32)
    core_of, pos2 = pre["core_of"], pre["pos2"]
    # device stored row r = p*T + t for slot t*128+p
    sl = np.arange(S)
    rowmap = (sl % P) * T + sl // P
    out[np.arange(N)] = slices[core_of, rowmap[pos2]]
    if n_timed:
        kernel.last_times = times
    return out


kernel.last_times = []


# revision 46
# speedup vs baseline: 1.0669x; 1.0480x over previous
"""Bass/Trainium2 kernel for 2-layer GAT (nn_GAT_58128087384143).

Strategy (8 NeuronCores, one SPMD NEFF):
  - Destination nodes are partitioned across the 8 cores, degree-sorted and
    assigned round-robin by rank so every core's tile t holds similarly
    sized ELL rows; one shared (core, slot) layout serves both layers'
    segment softmax / aggregation.
  - Every core computes the full "hext" node table (replicated):
    hext[row(n)] = [h(n) as bf16 | al_src(n)] where h = x @ W and
    al_src = x @ (W a_src) come out of one PE matmul per 128-node tile
    (phase A), stored p-major so writes are a few large descriptors.
  - al_dst is never gathered: each core computes it for its own dst slots
    with tiny PE matmuls against xS (host-permuted x, layer 1) or its own
    relu(out1).T slice (layer 2).
  - Per layer, per dst-tile group (phase C): dma_gather fetches the 256-byte
    hext rows of all in-edge sources (degree-bucketed ELL, padded slots point
    at sentinel rows whose al = -1e30 so exp() kills them), ACT computes
    leaky_relu logits (Lrelu) + exp with a fused row-sum (no max-subtraction:
    logits are provably < 15 for this model family), DVE does the broadcast
    multiply + k-reduction, ACT applies 1/denom, DVE adds bias.
  - dma_gather indices are int16, so each hext table is addressed through
    two 32768-row windows; edges whose source row lands in the overlap go to
    whichever side minimizes the per-tile ELL widths.
  - Between layers, relu(out1).T is AllGathered in column chunks so the
    collective overlaps the tail of phase C1 and hext2 construction (phase
    A2) proceeds per-chunk as data arrives.

kernel(**inputs) -> np.ndarray [50000, 64] float32.
"""

import numpy as np

P = 128
NCORES = 8
N = 50000
F_IN = 128
HID = 64
T = 49                 # dst tiles per core
S = T * P              # 6272 dst slots per core
CONCAT = NCORES * S    # 50176
NR1 = N + 3            # rows: 0=sent_neg, 1=sent_zero, 2..N+1 nodes, N+2=sent_neg_hi
NR2 = CONCAT + 3
WIN = 32768            # int16 gather window
HIB1 = NR1 - WIN       # hi window base row
HIB2 = NR2 - WIN
ROWW = 64              # fp32 elements per hext row (256 B); h is bf16
ALS = 32               # hext f32 col of al_src
ALD = 33               # hext f32 col of al_dst (written/used by layer 2 only)
NEGINF = -1.0e30
NEG_SLOPE = 0.2
RHSW = 128             # phase-A rhs width: [W | wa_src | wa_dst | 0...]
CHUNK = 4096           # phase-A input streaming chunk (cols)
STAGE_TILES = 32       # node-tiles per hext store
KCAP = 56              # max summed ELL width per gather group
CSPLIT = (21, 20, 8)   # collective chunks, in dst tiles (small tail)
FP8_EXCHANGE = True    # ship relu(out1) as fp8 e4m3 instead of bf16


# ----------------------------------------------------------------------------
# host-side graph preprocessing
# ----------------------------------------------------------------------------

def _cumcount(keys_sorted):
    n = keys_sorted.shape[0]
    if n == 0:
        return np.zeros(0, np.int64)
    first = np.ones(n, bool)
    first[1:] = keys_sorted[1:] != keys_sorted[:-1]
    idx = np.arange(n)
    start = np.maximum.accumulate(np.where(first, idx, 0))
    return idx - start


def _pack16(flat):
    """[n] int -> [128, n//16] int16: idx j at partition j%16, col j//16,
    replicated 8x down the partitions (one copy per Q7 core pair)."""
    n = flat.shape[0]
    assert n % 16 == 0
    block = flat.reshape(n // 16, 16).T.astype(np.int16)
    return np.tile(block, (8, 1))


def _window_k(mustlo, musthi, deg):
    """Per-tile optimal ELL widths given per-slot must-lo/must-hi/total
    degrees shaped [NCORES, T, P]. Returns K_lo[T], K_hi[T]."""
    A = mustlo.max(axis=(0, 2))
    B = musthi.max(axis=(0, 2))
    D = deg.max(axis=(0, 2))
    K_lo = np.maximum(A, 1)   # >=1 so dead slots have a sentinel column
    K_hi = np.maximum(B, D - K_lo)
    K_hi = np.maximum(K_hi, 0)
    return K_lo.astype(np.int64), K_hi.astype(np.int64)


def _order_score(key_cols, mustlo_d, musthi_d, deg_d):
    order = np.lexsort(key_cols)
    ml = np.zeros(CONCAT, np.int64)
    mh = np.zeros(CONCAT, np.int64)
    dg = np.zeros(CONCAT, np.int64)
    ml[:N] = mustlo_d[order]
    mh[:N] = musthi_d[order]
    dg[:N] = deg_d[order]
    ml = ml.reshape(S, NCORES).T.reshape(NCORES, T, P)
    mh = mh.reshape(S, NCORES).T.reshape(NCORES, T, P)
    dg = dg.reshape(S, NCORES).T.reshape(NCORES, T, P)
    K_lo, K_hi = _window_k(ml, mh, dg)
    return (K_lo + K_hi).sum() * P * NCORES, order


def _side_assign(dst, mustlo_e, musthi_e, K_lo_of_dst, K_hi_of_dst,
                 deg_d, mustlo_d):
    """Choose lo/hi side per edge (flex edges fill lo up to what K_hi can't
    absorb)."""
    flex_e = ~(mustlo_e | musthi_e)
    lo_cap = K_lo_of_dst - mustlo_d
    need_lo = deg_d - mustlo_d - K_hi_of_dst
    x_d = np.clip(need_lo, 0, np.maximum(lo_cap, 0))
    order = np.lexsort((~flex_e, dst))
    pos = _cumcount(dst[order])
    flexrank = np.full(dst.shape[0], 1 << 30, np.int64)
    flexrank[order] = np.where(flex_e[order], pos, 1 << 30)
    lo_e = mustlo_e | (flexrank < x_d[dst])
    return lo_e


def _make_groups(K_lo, K_hi, cap, csplit):
    """Greedy grouping of dst tiles into gather groups with summed width
    <= cap, never straddling collective-chunk boundaries (layer 1 uses the
    boundaries so chunk j's tiles finish before its AllGather; layer 2 just
    reuses the same grouping code with one big chunk)."""
    bounds = []
    t0 = 0
    for c in csplit:
        bounds.append((t0, t0 + c))
        t0 += c
    groups = []
    for (b0, b1) in bounds:
        t = b0
        while t < b1:
            kp = int(K_lo[t] + K_hi[t])
            ts = [t]
            t += 1
            while t < b1 and kp + int(K_lo[t] + K_hi[t]) <= cap:
                kp += int(K_lo[t] + K_hi[t])
                ts.append(t)
                t += 1
            groups.append(ts)
    return groups


def _build_ell(dst, row_of_edge, lo_e, core_of_dst, pos_of_dst,
               K_lo, K_hi, hib, sent_hi_val, dead, groups):
    """Build per-core packed int16 index arrays for the per-group gathers.

    Group g covers tiles ts: one lo index block (tiles concatenated k-major)
    then one hi block, so each group needs two dma_gather calls."""
    core_e = core_of_dst[dst]
    pos_e = pos_of_dst[dst]
    side_e = (~lo_e).astype(np.int64)
    order = np.lexsort((side_e, pos_e, core_e))
    key = ((core_e[order] * S + pos_e[order]) << 1) | side_e[order]
    cc = _cumcount(key)

    KLM = int(K_lo.max())
    KHM = int(max(1, K_hi.max()))
    ell_lo = np.zeros((NCORES, S, KLM), np.int64)            # sent_neg = row 0
    ell_hi = np.full((NCORES, S, KHM), sent_hi_val, np.int64)
    oe = order
    lo_sel = lo_e[oe]
    ell_lo[core_e[oe][lo_sel], pos_e[oe][lo_sel], cc[lo_sel]] = \
        row_of_edge[oe][lo_sel]
    hi_sel = ~lo_sel
    ell_hi[core_e[oe][hi_sel], pos_e[oe][hi_sel], cc[hi_sel]] = \
        row_of_edge[oe][hi_sel] - hib
    # dead slots: first lo column -> sent_zero (row 1) so denom = 1, out = 0
    dc, dp = np.nonzero(dead)
    ell_lo[dc, dp, 0] = 1

    packs = [[] for _ in range(NCORES)]
    ginfo = []   # (idx_lo_off, n_lo, idx_hi_off, n_hi, kp) per group
    tinfo = []   # (group, lo_off, kl, hi_off, kh) per tile, tile-indexed
    col = 0
    tmap = {}
    for gi, ts in enumerate(groups):
        kls = [int(K_lo[t]) for t in ts]
        khs = [int(K_hi[t]) for t in ts]
        n_lo, n_hi = sum(kls), sum(khs)
        kp = n_lo + n_hi
        lo_off = col
        col += 8 * n_lo
        hi_off = col
        col += 8 * n_hi
        ginfo.append((lo_off, n_lo, hi_off, n_hi, kp))
        run = 0
        for i, t in enumerate(ts):
            tmap[t] = (gi, run, kls[i], n_lo + sum(khs[:i]), khs[i])
            run += kls[i]
        for c in range(NCORES):
            blks = [ell_lo[c, t * P:(t + 1) * P, :int(K_lo[t])].T.reshape(-1)
                    for t in ts]
            packs[c].append(_pack16(np.concatenate(blks)))
            if n_hi:
                blks = [ell_hi[c, t * P:(t + 1) * P, :int(K_hi[t])].T.reshape(-1)
                        for t in ts]
                packs[c].append(_pack16(np.concatenate(blks)))
    tinfo = [tmap[t] for t in range(T)]
    idx = np.stack([np.concatenate(p, axis=1) for p in packs])  # [NC,128,C]
    return np.ascontiguousarray(idx), (ginfo, tinfo, groups), col


def _rowmap_pmajor(total):
    """DRAM row offset for each sequential stream position, matching phase
    A1's p-major stage stores over CHUNK-column spans."""
    rm = np.empty(total, np.int64)
    for base in range(0, total, CHUNK):
        cols = min(CHUNK, total - base)
        idx = np.arange(cols)
        if cols % P == 0:
            ntile = cols // P
            rm[base:base + cols] = base + (idx % P) * ntile + idx // P
        else:
            rm[base:base + cols] = base + idx
    return rm


def _rowmap_csplit():
    """Per-core row offset of each slot for phase A2's per-collective-chunk
    p-major stores."""
    rm = np.empty(S, np.int64)
    base = 0
    for c in CSPLIT:
        cols = c * P
        idx = np.arange(cols)
        rm[base:base + cols] = base + (idx % P) * c + idx // P
        base += cols
    return rm


def _preprocess(edge_index):
    src = np.concatenate([edge_index[0].astype(np.int64), np.arange(N)])
    dst = np.concatenate([edge_index[1].astype(np.int64), np.arange(N)])
    deg_d = np.bincount(dst, minlength=N)
    outdeg = np.bincount(src, minlength=N)

    # ---------- layer-1 row placement (via host x permutation) ----------
    # top-out-degree nodes go to rows in the two-window overlap so the most
    # edges become side-flexible; the rest alternate lo/hi by out-degree.
    rows_of_pos = _rowmap_pmajor(N) + 2
    ov = (rows_of_pos >= HIB1) & (rows_of_pos < WIN)
    over_pos = np.where(ov)[0]
    lo_pos = np.where(rows_of_pos < HIB1)[0]
    hi_pos = np.where(rows_of_pos >= WIN)[0]
    pos_of_node = np.arange(N)
    xposinv = np.empty(N, np.int64)
    xposinv[pos_of_node] = np.arange(N)      # stream position -> node
    rowmap1 = rows_of_pos[pos_of_node]       # hext1 row of node
    row1 = rowmap1[src]
    mustlo1_e = row1 < HIB1
    musthi1_e = row1 >= WIN
    mustlo1_d = np.bincount(dst[mustlo1_e], minlength=N)
    musthi1_d = np.bincount(dst[musthi1_e], minlength=N)

    blk_map = _rowmap_csplit()
    cands = [
        (-deg_d, -mustlo1_d),
        (-musthi1_d, -mustlo1_d),
        (-mustlo1_d, -musthi1_d),
        (-(mustlo1_d + musthi1_d), -deg_d),
        (-deg_d, -(mustlo1_d - musthi1_d)),
        (-np.maximum(mustlo1_d, musthi1_d), -deg_d),
    ]
    best = None
    for kc in cands:
        score, order = _order_score(kc, mustlo1_d, musthi1_d, deg_d)
        if best is None or score < best[0]:
            best = (score, order)
    slots1, order1 = best
    rank1 = np.empty(N, np.int64)
    rank1[order1] = np.arange(N)
    core_of = rank1 % NCORES
    pos = rank1 // NCORES
    dead = np.ones((NCORES, S), np.uint8)
    dead[core_of, pos] = 0

    ml = np.zeros(CONCAT, np.int64); mh = np.zeros(CONCAT, np.int64)
    dg = np.zeros(CONCAT, np.int64)
    ml[:N] = mustlo1_d[order1]; mh[:N] = musthi1_d[order1]
    dg[:N] = deg_d[order1]
    K1_lo, K1_hi = _window_k(ml.reshape(S, NCORES).T.reshape(NCORES, T, P),
                             mh.reshape(S, NCORES).T.reshape(NCORES, T, P),
                             dg.reshape(S, NCORES).T.reshape(NCORES, T, P))
    lo1_e = _side_assign(dst, mustlo1_e, musthi1_e, K1_lo[pos // P],
                         K1_hi[pos // P], deg_d, mustlo1_d)
    groups1 = _make_groups(K1_lo, K1_hi, KCAP, CSPLIT)
    idx1, offs1, C1 = _build_ell(dst, row1, lo1_e, core_of, pos,
                                 K1_lo, K1_hi, HIB1, NR1 - 1 - HIB1, dead,
                                 groups1)

    # ---------- layer 2 (own dst ordering; source rows are concat slots) --
    crow = core_of * S + pos
    rowmap2cat = ((crow // S) * S + blk_map[crow % S]) + 2
    r2 = rowmap2cat[src]
    mustlo2_e = r2 < HIB2
    musthi2_e = r2 >= WIN
    mustlo2_d = np.bincount(dst[mustlo2_e], minlength=N)
    musthi2_d = np.bincount(dst[musthi2_e], minlength=N)

    cands2 = [
        (-deg_d, -mustlo2_d, core_of),
        (-musthi2_d, -mustlo2_d, core_of),
        (-mustlo2_d, -musthi2_d, core_of),
        (-deg_d, -(mustlo2_d - musthi2_d), core_of),
        (-(mustlo2_d + musthi2_d), -deg_d, core_of),
        (-np.maximum(mustlo2_d, musthi2_d), -deg_d, core_of),
    ]
    best2 = None
    for kc in cands2:
        o2 = np.lexsort(kc)
        p2 = np.empty(N, np.int64)
        p2[o2] = _cumcount(core_of[o2])
        ml = np.zeros((NCORES, S), np.int64)
        mh = np.zeros((NCORES, S), np.int64)
        dg2 = np.zeros((NCORES, S), np.int64)
        ml[core_of, p2] = mustlo2_d
        mh[core_of, p2] = musthi2_d
        dg2[core_of, p2] = deg_d
        klo, khi = _window_k(ml.reshape(NCORES, T, P),
                             mh.reshape(NCORES, T, P),
                             dg2.reshape(NCORES, T, P))
        score = int((klo + khi).sum())
        if best2 is None or score < best2[0]:
            best2 = (score, p2, klo, khi)
    _, pos2, K2_lo, K2_hi = best2
    slots2 = int((K2_lo + K2_hi).sum()) * P * NCORES
    dead2 = np.ones((NCORES, S), np.uint8)
    dead2[core_of, pos2] = 0

    lo2_e = _side_assign(dst, mustlo2_e, musthi2_e, K2_lo[pos2 // P],
                         K2_hi[pos2 // P], deg_d, mustlo2_d)
    groups2 = _make_groups(K2_lo, K2_hi, KCAP, (T,))
    idx2, offs2, C2 = _build_ell(dst, r2, lo2_e, core_of, pos2,
                                 K2_lo, K2_hi, HIB2, NR2 - 1 - HIB2, dead2,
                                 groups2)

    # ---------- per-dst-row al_dst2 gather (layer 2 phase B) ----------
    rect_lo = np.zeros((NCORES, S), np.int64)
    rect_hi = np.full((NCORES, S), NR2 - 1 - HIB2, np.int64)
    mask_hi = np.zeros((NCORES, S), np.uint8)
    r = rowmap2cat
    is_lo = r < WIN
    rect_lo[core_of[is_lo], pos2[is_lo]] = r[is_lo]
    ih = ~is_lo
    rect_hi[core_of[ih], pos2[ih]] = r[ih] - HIB2
    mask_hi[core_of[ih], pos2[ih]] = 1
    # dead slots: lo sentinel-zero row so al_dst = 0
    rect_lo[dead2 > 0] = 1
    didx2 = np.stack([np.concatenate(
        [_pack16(rect_lo[c]), _pack16(rect_hi[c])], axis=1)
        for c in range(NCORES)])
    mh2 = np.ascontiguousarray(np.stack(
        [mask_hi[c].reshape(T, P).T for c in range(NCORES)]))

    stats = dict(slots1=int(slots1), slots2=int(slots2),
                 edges=int(dst.shape[0]),
                 pad1=float(slots1) / dst.shape[0],
                 pad2=float(slots2) / dst.shape[0])
    return dict(idx1=idx1, offs1=offs1, C1=C1, K1_lo=K1_lo, K1_hi=K1_hi,
                rowmap1=rowmap1, rowmap2cat=rowmap2cat, crow=crow,
                xposinv=xposinv,
                idx2=idx2, offs2=offs2, C2=C2, K2_lo=K2_lo, K2_hi=K2_hi,
                didx2=didx2, mh2=mh2,
                core_of=core_of, pos=pos, pos2=pos2, blk_map=blk_map,
                stats=stats)


# ----------------------------------------------------------------------------
# device kernel
# ----------------------------------------------------------------------------

def _build_nc(pre):
    import concourse.bass as bass
    import concourse.mybir as mybir
    import concourse.tile as tile
    from concourse import bacc
    from concourse.masks import make_identity

    f32 = mybir.dt.float32
    bf16 = mybir.dt.bfloat16
    i16 = mybir.dt.int16
    x8 = mybir.dt.float8e4 if FP8_EXCHANGE else bf16
    AF = mybir.ActivationFunctionType
    OP = mybir.AluOpType
    AX = mybir.AxisListType

    offs1, offs2 = pre["offs1"], pre["offs2"]
    C1, C2 = pre["C1"], pre["C2"]

    nc = bacc.Bacc("TRN2", num_devices=NCORES, target_bir_lowering=False)

    xT = nc.dram_tensor("xT", [F_IN, N], bf16, kind="ExternalInput")
    xS = nc.dram_tensor("xS", [F_IN, S], bf16, kind="ExternalInput")
    rhs1 = nc.dram_tensor("rhs1", [F_IN, RHSW], f32, kind="ExternalInput")
    rhs2 = nc.dram_tensor("rhs2", [HID, RHSW], f32, kind="ExternalInput")
    wad1 = nc.dram_tensor("wad1", [F_IN, 1], bf16, kind="ExternalInput")
    b1r = nc.dram_tensor("b1r", [P, HID], f32, kind="ExternalInput")
    b2r = nc.dram_tensor("b2r", [P, HID], f32, kind="ExternalInput")
    idx1 = nc.dram_tensor("idx1", [P, C1], i16, kind="ExternalInput")
    idx2 = nc.dram_tensor("idx2", [P, C2], i16, kind="ExternalInput")
    didx2 = nc.dram_tensor("didx2", [P, 2 * (S // 16)], i16,
                           kind="ExternalInput")
    mh2 = nc.dram_tensor("mh2", [P, T], mybir.dt.uint8, kind="ExternalInput")
    out2 = nc.dram_tensor("out", [S, HID], f32, kind="ExternalOutput")

    hext1 = nc.dram_tensor("hext1", [NR1, ROWW], f32, kind="Internal")
    hext2 = nc.dram_tensor("hext2", [NR2, ROWW], f32, kind="Internal")
    o1c = [nc.dram_tensor(f"o1c{j}", [HID, CSPLIT[j] * P], x8, kind="Internal")
           for j in range(len(CSPLIT))]
    ag = [nc.dram_tensor(f"ag{j}", [NCORES, HID, CSPLIT[j] * P], x8,
                         kind="Internal", addr_space="Shared")
          for j in range(len(CSPLIT))]

    KMAX = int(max(max(g[4] for g in offs1[0]), max(g[4] for g in offs2[0])))
    NTGMAX = max(max(len(ts) for ts in offs1[2]),
                 max(len(ts) for ts in offs2[2]))

    with tile.TileContext(nc) as tc:
        with tc.tile_pool(name="const", bufs=1) as cp:
            rhs1_sb = cp.tile([F_IN, RHSW], bf16)
            nc.gpsimd.dma_start(out=rhs1_sb[:], in_=rhs1[:, :])
            rhs2_sb = cp.tile([HID, RHSW], bf16)
            nc.gpsimd.dma_start(out=rhs2_sb[:], in_=rhs2[:, :])
            wad1_sb = cp.tile([F_IN, 1], bf16)
            nc.gpsimd.dma_start(out=wad1_sb[:], in_=wad1[:, :])
            b1_sb = cp.tile([P, HID], f32)
            nc.sync.dma_start(out=b1_sb[:], in_=b1r[:, :])
            b2_sb = cp.tile([P, HID], f32)
            nc.sync.dma_start(out=b2_sb[:], in_=b2r[:, :])
            ident = cp.tile([P, P], f32)
            make_identity(nc, ident[:])
            xS_sb = cp.tile([F_IN, S], bf16)
            nc.sync.dma_start(out=xS_sb[:], in_=xS[:, :])
            o1T_sb = cp.tile([HID, S], bf16)
            ald1 = cp.tile([P, T], f32)
            ald1_02 = cp.tile([P, T], f32)
            # sentinel rows: row0 al=-1e30 (pad), row1 al=0 (dead slots)
            sent = cp.tile([2, ROWW], f32)
            nc.vector.memset(sent[:], 0.0)
            nc.vector.memset(sent[0:1, ALS:ALS + 1], NEGINF)
            # index tables: load up front so they never queue behind
            # collective-gated DMAs
            idx1_sb = cp.tile([P, C1], i16)
            nc.sync.dma_start(out=idx1_sb[:], in_=idx1[:, :])
            idx2_sb = cp.tile([P, C2], i16)
            nc.sync.dma_start(out=idx2_sb[:], in_=idx2[:, :])
            didx2_sb = cp.tile([P, 2 * (S // 16)], i16)
            nc.sync.dma_start(out=didx2_sb[:], in_=didx2[:, :])
            mh2_sb = cp.tile([P, T], mybir.dt.uint8)
            nc.sync.dma_start(out=mh2_sb[:], in_=mh2[:, :])

            def phase_b1():
                """al_dst1 for this core's own dst slots via tiny matmuls
                against the host-permuted x (no gather needed)."""
                with tc.tile_pool(name="pb1", bufs=1, space="PSUM") as pb:
                    ps = pb.tile([P, T], f32, space="PSUM")
                    for t in range(T):
                        nc.tensor.matmul(out=ps[:, t:t + 1],
                                         lhsT=xS_sb[:, t * P:(t + 1) * P],
                                         rhs=wad1_sb[:],
                                         start=True, stop=True,
                                         skip_group_check=True)
                    nc.scalar.activation(out=ald1[:], in_=ps[:], func=AF.Copy)
                    nc.vector.tensor_scalar(out=ald1_02[:], in0=ald1[:],
                                            scalar1=NEG_SLOPE, scalar2=None,
                                            op0=OP.mult)

            def phase_a(layer):
                assert layer == 1
                hext = hext1
                rhs_sb = rhs1_sb
                kdim = F_IN
                with tc.tile_pool(name=f"pa{layer}", bufs=3) as pa, \
                     tc.tile_pool(name=f"pap{layer}", bufs=6, space="PSUM") as pp:
                    nc.sync.dma_start(out=hext[0:2, :], in_=sent[:])
                    nc.sync.dma_start(out=hext[NR1 - 1:NR1, :],
                                      in_=sent[0:1, :])

                    spans = [(c0, min(CHUNK, N - c0), 0, 2 + c0)
                             for c0 in range(0, N, CHUNK)]
                    for c0, cols, blk, rowbase in spans:
                        in_sb = pa.tile([kdim, CHUNK], bf16, tag="pa_in")
                        nc.sync.dma_start(out=in_sb[:, 0:cols],
                                          in_=xT[:, c0:c0 + cols])
                        ntile = (cols + P - 1) // P
                        stage = pa.tile([P, STAGE_TILES, ROWW], f32, tag="pa_st")
                        stage_bf = stage[:].bitcast(bf16)
                        QUAD = 4
                        nt = 0
                        while nt < ntile:
                            q = min(QUAD, ntile - nt)
                            rows = [min(P, cols - (nt + i) * P)
                                    for i in range(q)]
                            if rows[0] == P:
                                while q > 1 and rows[q - 1] < P:
                                    q -= 1
                            else:
                                q = 1
                            r = rows[0] if q == 1 else P
                            ps = pp.tile([P, QUAD, RHSW], f32, space="PSUM")
                            for i in range(q):
                                nc.tensor.matmul(
                                    out=ps[0:r, i, :],
                                    lhsT=in_sb[:, (nt + i) * P:
                                               (nt + i) * P + r],
                                    rhs=rhs_sb[:],
                                    start=True, stop=True,
                                    skip_group_check=True)
                            nc.scalar.activation(
                                out=stage_bf[0:r, nt:nt + q, 0:HID],
                                in_=ps[0:r, 0:q, 0:HID], func=AF.Copy)
                            nc.vector.tensor_copy(
                                out=stage[0:r, nt:nt + q, ALS:ALS + 1],
                                in_=ps[0:r, 0:q, HID:HID + 1])
                            nt += q
                        full = cols // P
                        rem = cols - full * P
                        if rem == 0:
                            nc.sync.dma_start(
                                out=hext[rowbase:rowbase + cols, :]
                                .rearrange("(p n) w -> p n w", p=P),
                                in_=stage[:, 0:full, :])
                        else:
                            if full:
                                nc.sync.dma_start(
                                    out=hext[rowbase:rowbase + full * P, :]
                                    .rearrange("(n p) w -> p n w", p=P),
                                    in_=stage[:, 0:full, :])
                            nc.sync.dma_start(
                                out=hext[rowbase + full * P:
                                         rowbase + full * P + rem, :]
                                .rearrange("(n p) w -> p n w", p=rem),
                                in_=stage[0:rem, full:full + 1, :])

            def phase_c(layer, bc, gp, b1p, bp, on_chunk_done=None):
                hext = hext1 if layer == 1 else hext2
                hib = HIB1 if layer == 1 else HIB2
                offs = offs1 if layer == 1 else offs2
                idx_t = idx1 if layer == 1 else idx2
                cdim = C1 if layer == 1 else C2
                b_sb = b1_sb if layer == 1 else b2_sb

                src_lo = hext[0:WIN, :]
                src_hi = hext[hib:hib + WIN, :]
                ginfo, tinfo, groups = offs

                idx_sb = idx1_sb if layer == 1 else idx2_sb
                if layer == 1:
                    ald = ald1
                    ald02 = ald1_02
                else:
                    o2_sb = b1p.tile([P, T, HID], f32)
                    # al_dst2 per own dst slot: gather hext2 rows, read ALD
                    Gd_lo = b1p.tile([P, T, ROWW], f32)
                    nc.gpsimd.dma_gather(
                        out_ap=Gd_lo[:], in_ap=src_lo,
                        idxs_ap=didx2_sb[:, 0:S // 16],
                        num_idxs=S, num_idxs_reg=S, elem_size=ROWW,
                        single_packet=False)
                    Gd_hi = b1p.tile([P, T, ROWW], f32)
                    nc.gpsimd.dma_gather(
                        out_ap=Gd_hi[:], in_ap=src_hi,
                        idxs_ap=didx2_sb[:, S // 16:2 * (S // 16)],
                        num_idxs=S, num_idxs_reg=S, elem_size=ROWW,
                        single_packet=False)
                    ald = b1p.tile([P, T], f32)
                    nc.vector.tensor_copy(out=ald[:], in_=Gd_lo[:, :, ALD])
                    nc.vector.copy_predicated(out=ald[:], mask=mh2_sb[:],
                                              data=Gd_hi[:, :, ALD])
                    ald02 = b1p.tile([P, T], f32)
                    nc.vector.tensor_scalar(out=ald02[:], in0=ald[:],
                                            scalar1=NEG_SLOPE, scalar2=None,
                                            op0=OP.mult)

                chunk_end = []
                t0 = 0
                for c in CSPLIT:
                    chunk_end.append(t0 + c)
                    t0 += c
                pending = []

                def emit_exchange(j):
                    nc.gpsimd.collective_compute(
                        kind="AllGather", op=OP.bypass,
                        replica_groups=[list(range(NCORES))],
                        ins=[o1c[j][:, :]], outs=[ag[j][:, :, :]])
                    # hext2 build for the PREVIOUS chunk: its collective has
                    # landed by now, so the SP queue never parks on an
                    # unfinished AllGather in front of later o1c stores.
                    if on_chunk_done is not None and j > 0:
                        on_chunk_done(j - 1)

                for gi, ts in enumerate(groups):
                    ilo, n_lo, ihi, n_hi, kp = ginfo[gi]
                    G = gp.tile([P, KMAX, ROWW], f32, tag="G")
                    nc.gpsimd.dma_gather(
                        out_ap=G[:, 0:n_lo, :], in_ap=src_lo,
                        idxs_ap=idx_sb[:, ilo:ilo + 8 * n_lo],
                        num_idxs=P * n_lo, num_idxs_reg=P * n_lo,
                        elem_size=ROWW, single_packet=False)
                    if n_hi:
                        nc.gpsimd.dma_gather(
                            out_ap=G[:, n_lo:kp, :], in_ap=src_hi,
                            idxs_ap=idx_sb[:, ihi:ihi + 8 * n_hi],
                            num_idxs=P * n_hi, num_idxs_reg=P * n_hi,
                            elem_size=ROWW, single_packet=False)
                    Gh = G[:].bitcast(bf16)
                    ntg = len(ts)
                    exg = bc.tile([P, KMAX], f32, tag="exg")
                    e0 = bc.tile([P, KMAX], f32, tag="e0")
                    e1 = bc.tile([P, KMAX], f32, tag="e1")
                    den = bc.tile([P, 2, NTGMAX], f32, tag="den")
                    rec = bc.tile([P, NTGMAX], f32, tag="rec")
                    any_hi = any(tinfo[t][4] for t in ts)
                    if any_hi:
                        nc.vector.memset(den[:, 1, 0:ntg], 0.0)
                    reds = []
                    # pass 1: ex = exp(leaky_relu(al_src + al_dst)) in group
                    # layout; per-range row-sums -> den.  No max subtraction:
                    # logits are bounded (~15) for this model family.
                    for ti, t in enumerate(ts):
                        _, lo_off, kl, hi_off, kh = tinfo[t]
                        ad = ald[:, t:t + 1]
                        ad02 = ald02[:, t:t + 1]
                        for si, (o, k) in enumerate(((lo_off, kl),
                                                     (hi_off, kh))):
                            if k == 0:
                                continue
                            # leaky_relu(x + ad) = max(x + ad, 0.2x + 0.2ad)
                            nc.scalar.activation(
                                out=e0[:, o:o + k], in_=G[:, o:o + k, ALS],
                                func=AF.Identity, bias=ad, scale=1.0)
                            nc.scalar.activation(
                                out=e1[:, o:o + k], in_=G[:, o:o + k, ALS],
                                func=AF.Identity, bias=ad02,
                                scale=NEG_SLOPE)
                            nc.vector.tensor_tensor(
                                out=e1[:, o:o + k], in0=e0[:, o:o + k],
                                in1=e1[:, o:o + k], op=OP.max)
                            nc.scalar.activation(
                                out=exg[:, o:o + k], in_=e1[:, o:o + k],
                                func=AF.Exp,
                                accum_out=den[:, si, ti:ti + 1])
                        # weighted sum can start before the denominators are
                        # merged — only the final scale needs 1/den
                        kt = kl + kh
                        prod = bc.tile([P, KMAX, HID], bf16, tag="prod")
                        for (o, k, d0) in ((lo_off, kl, 0),
                                           (hi_off, kh, kl)):
                            if k == 0:
                                continue
                            nc.vector.tensor_tensor(
                                out=prod[:, d0:d0 + k, :],
                                in0=Gh[:, o:o + k, 0:HID],
                                in1=exg[:, o:o + k, None]
                                .to_broadcast([P, k, HID]),
                                op=OP.mult)
                        red = bc.tile([P, HID], f32, tag=f"red{ti}")
                        nc.vector.tensor_reduce(
                            out=red[:], in_=prod[:, 0:kt, :].rearrange(
                                "p k f -> p f k"),
                            axis=AX.X, op=OP.add)
                        reds.append(red)
                    if any_hi:
                        nc.vector.tensor_tensor(out=den[:, 0, 0:ntg],
                                                in0=den[:, 0, 0:ntg],
                                                in1=den[:, 1, 0:ntg],
                                                op=OP.add)
                    nc.vector.reciprocal(out=rec[:, 0:ntg],
                                         in_=den[:, 0, 0:ntg])
                    # pass 2: normalize + bias + store per tile
                    for ti, t in enumerate(ts):
                        outt = bc.tile([P, HID], f32, tag="outt")
                        nc.scalar.activation(out=outt[:], in_=reds[ti][:],
                                             func=AF.Copy,
                                             scale=rec[:, ti:ti + 1])
                        if layer == 1:
                            nc.vector.tensor_tensor(out=outt[:], in0=outt[:],
                                                    in1=b_sb[:], op=OP.add)
                            psT = bp.tile([HID, P], f32, space="PSUM")
                            nc.tensor.transpose(out=psT[:], in_=outt[:],
                                                identity=ident[:])
                            nc.scalar.activation(
                                out=o1T_sb[:, t * P:(t + 1) * P],
                                in_=psT[:], func=AF.Relu)
                        else:
                            nc.vector.tensor_tensor(out=o2_sb[:, t, :],
                                                    in0=outt[:], in1=b_sb[:],
                                                    op=OP.add)
                    if layer == 1 and ts[-1] + 1 in chunk_end:
                        j = chunk_end.index(ts[-1] + 1)
                        cbase = (chunk_end[j - 1] if j else 0) * P
                        cw = CSPLIT[j] * P
                        if FP8_EXCHANGE:
                            o1x = b1p.tile([HID, S], x8, tag="o1x")
                            nc.vector.tensor_copy(
                                out=o1x[:, cbase:cbase + cw],
                                in_=o1T_sb[:, cbase:cbase + cw])
                            nc.sync.dma_start(out=o1c[j][:, :],
                                              in_=o1x[:, cbase:cbase + cw])
                        else:
                            nc.sync.dma_start(out=o1c[j][:, :],
                                              in_=o1T_sb[:, cbase:cbase + cw])
                        # the collective itself is emitted two groups later:
                        # it parks Pool SEQ until o1c lands, so give the next
                        # chunk's gather preps a head start in the queue
                        pending.append((gi + 2, j))
                    while pending and pending[0][0] <= gi:
                        emit_exchange(pending.pop(0)[1])

                while pending:
                    emit_exchange(pending.pop(0)[1])
                if layer == 1 and on_chunk_done is not None:
                    on_chunk_done(len(CSPLIT) - 1)

                if layer == 2:
                    nc.sync.dma_start(
                        out=out2[:, :].rearrange("(p t) f -> p t f", p=P),
                        in_=o2_sb[:])

            def phase_a2_chunk(j, pa, pp):
                """hext2 rows for collective chunk j, all 8 source blocks."""
                cbase = sum(CSPLIT[:j]) * P
                cols = CSPLIT[j] * P
                ntile = CSPLIT[j]
                for blk in range(NCORES):
                    rowbase = 2 + blk * S + cbase
                    in_sb = pa.tile([HID, CHUNK], bf16, tag="pa_in")
                    if FP8_EXCHANGE:
                        raw = pa.tile([HID, CHUNK], x8, tag="pa_raw")
                        nc.sync.dma_start(out=raw[:, 0:cols],
                                          in_=ag[j][blk, :, :])
                        nc.vector.tensor_copy(out=in_sb[:, 0:cols],
                                              in_=raw[:, 0:cols])
                    else:
                        nc.sync.dma_start(out=in_sb[:, 0:cols],
                                          in_=ag[j][blk, :, :])
                    stage = pa.tile([P, STAGE_TILES, ROWW], f32, tag="pa_st")
                    stage_bf = stage[:].bitcast(bf16)
                    QUAD = 4
                    nt = 0
                    while nt < ntile:
                        q = min(QUAD, ntile - nt)
                        ps = pp.tile([P, QUAD, RHSW], f32, space="PSUM")
                        for i in range(q):
                            nc.tensor.matmul(
                                out=ps[:, i, :],
                                lhsT=in_sb[:, (nt + i) * P:(nt + i + 1) * P],
                                rhs=rhs2_sb[:],
                                start=True, stop=True,
                                skip_group_check=True)
                        nc.scalar.activation(
                            out=stage_bf[:, nt:nt + q, 0:HID],
                            in_=ps[:, 0:q, 0:HID], func=AF.Copy)
                        nc.vector.tensor_copy(
                            out=stage[:, nt:nt + q, ALS:ALD + 1],
                            in_=ps[:, 0:q, HID:HID + 2])
                        nt += q
                    nc.sync.dma_start(
                        out=hext2[rowbase:rowbase + cols, :]
                        .rearrange("(p n) w -> p n w", p=P),
                        in_=stage[:, 0:ntile, :])

            phase_a(1)
            phase_b1()
            # layer-1 aggregation with inline chunked exchange + hext2 build
            with tc.tile_pool(name="bc1", bufs=3) as bc1t, \
                 tc.tile_pool(name="gp1", bufs=4) as gp1t, \
                 tc.tile_pool(name="one1", bufs=1) as b1p1, \
                 tc.tile_pool(name="bp1", bufs=2, space="PSUM") as bp1t, \
                 tc.tile_pool(name="pa2", bufs=3) as pa2t, \
                 tc.tile_pool(name="pap2", bufs=6, space="PSUM") as pp2t:
                nc.sync.dma_start(out=hext2[0:2, :], in_=sent[:])
                nc.sync.dma_start(out=hext2[NR2 - 1:NR2, :], in_=sent[0:1, :])
                phase_c(1, bc1t, gp1t, b1p1, bp1t,
                        on_chunk_done=lambda j: phase_a2_chunk(j, pa2t, pp2t))
            with tc.tile_pool(name="bc2", bufs=3) as bc2t, \
                 tc.tile_pool(name="gp2", bufs=4) as gp2t, \
                 tc.tile_pool(name="one2", bufs=1) as b1p2, \
                 tc.tile_pool(name="bp2", bufs=2, space="PSUM") as bp2t:
                phase_c(2, bc2t, gp2t, b1p2, bp2t)

    nc.compile()
    return nc


# ----------------------------------------------------------------------------
# runner (caches compiled executable; reuses device-resident inputs)
# ----------------------------------------------------------------------------

_CACHE = {}


def _make_inputs(x, edge_index, W1, a_src1, a_dst1, b1, W2, a_src2, a_dst2, b2,
                 pre):
    import ml_dtypes
    xT = np.ascontiguousarray(x[pre["xposinv"]].T.astype(ml_dtypes.bfloat16))
    r1 = np.zeros((F_IN, RHSW), np.float32)
    r1[:, 0:HID] = W1
    r1[:, HID] = W1 @ a_src1
    r2 = np.zeros((HID, RHSW), np.float32)
    r2[:, 0:HID] = W2
    r2[:, HID] = W2 @ a_src2
    r2[:, HID + 1] = W2 @ a_dst2
    w1d = (W1 @ a_dst1).astype(ml_dtypes.bfloat16)[:, None]
    b1rep = np.tile(b1[None, :], (P, 1)).astype(np.float32)
    b2rep = np.tile(b2[None, :], (P, 1)).astype(np.float32)
    core_of, pos = pre["core_of"], pre["pos"]
    xs_all = np.zeros((NCORES, F_IN, S), np.float32)
    xs_all[core_of, :, pos] = x          # [N,F] rows scattered
    xs_all = np.ascontiguousarray(xs_all).astype(ml_dtypes.bfloat16)
    in_maps = []
    for c in range(NCORES):
        in_maps.append(dict(
            xT=xT, xS=np.ascontiguousarray(xs_all[c]), rhs1=r1, rhs2=r2,
            wad1=w1d, b1r=b1rep, b2r=b2rep,
            idx1=np.ascontiguousarray(pre["idx1"][c]),
            idx2=np.ascontiguousarray(pre["idx2"][c]),
            didx2=np.ascontiguousarray(pre["didx2"][c]),
            mh2=np.ascontiguousarray(pre["mh2"][c]),
        ))
    return in_maps


def _get_runner(pre):
    if "runner" in _CACHE:
        return _CACHE["runner"]

    import jax
    import numpy as _np
    from jax.sharding import Mesh, PartitionSpec
    from jax.experimental.shard_map import shard_map
    import concourse.mybir as mybir
    from concourse import bass2jax
    from concourse.bass2jax import _bass_exec_p, install_neuronx_cc_hook

    nc = _CACHE["nc"]
    install_neuronx_cc_hook()

    partition_name = (nc.partition_id_tensor.name
                      if nc.partition_id_tensor else None)
    in_names, out_names, out_avals, zero_shapes = [], [], [], []
    for alloc in nc.m.functions[0].allocations:
        if not isinstance(alloc, mybir.MemoryLocationSet):
            continue
        name = alloc.memorylocations[0].name
        if alloc.kind == "ExternalInput":
            if name != partition_name:
                in_names.append(name)
        elif alloc.kind == "ExternalOutput":
            out_names.append(name)
            shape = tuple(alloc.tensor_shape)
            dtype = mybir.dt.np(alloc.dtype)
            out_avals.append(jax.core.ShapedArray(shape, dtype))
            zero_shapes.append((shape, dtype))
    n_params = len(in_names)
    all_names = in_names + out_names
    if partition_name is not None:
        all_names.append(partition_name)

    import jax.numpy as jnp

    def _body(*args):
        operands = list(args)
        if partition_name is not None:
            operands.append(bass2jax.partition_id_tensor())
        return tuple(_bass_exec_p.bind(
            *operands, out_avals=tuple(out_avals), in_names=tuple(all_names),
            out_names=tuple(out_names), lowering_input_output_aliases=(),
            sim_require_finite=True, sim_require_nnan=True, nc=nc))

    devices = jax.devices()[:NCORES]
    mesh = Mesh(_np.asarray(devices), ("core",))
    n_outs = len(out_names)
    in_specs = (PartitionSpec("core"),) * (n_params + n_outs)
    out_specs = (PartitionSpec("core"),) * n_outs
    donate = tuple(range(n_params, n_params + n_outs))
    sharded = jax.jit(
        shard_map(_body, mesh=mesh, in_specs=in_specs, out_specs=out_specs,
                  check_rep=False),
        donate_argnums=donate, keep_unused=True)

    from jax.sharding import NamedSharding
    zsharding = NamedSharding(mesh, PartitionSpec("core"))
    zeros_fn = jax.jit(
        lambda: tuple(jnp.zeros((NCORES * sh[0], *sh[1:]), dt)
                      for sh, dt in zero_shapes),
        out_shardings=(zsharding,) * n_outs)

    def run(in_maps, n_timed=0):
        concat_in = [
            _np.concatenate([_np.asarray(in_maps[c][nm]) for c in range(NCORES)],
                            axis=0)
            for nm in in_names]
        shardings = [NamedSharding(mesh, PartitionSpec("core"))] * n_params
        dev_in = [jax.device_put(a, s) for a, s in zip(concat_in, shardings)]

        outs = sharded(*dev_in, *zeros_fn())
        for o in outs:
            o.block_until_ready()
        times = []
        if n_timed:
            import time as _t
            for _ in range(n_timed):
                z = zeros_fn()
                for zz in z:
                    zz.block_until_ready()
                t0 = _t.perf_counter()
                outs2 = sharded(*dev_in, *z)
                for o in outs2:
                    o.block_until_ready()
                times.append(_t.perf_counter() - t0)
        result = {}
        for i, nm in enumerate(out_names):
            arr = _np.asarray(outs[i]).reshape(NCORES, *out_avals[i].shape)
            result[nm] = arr
        return result, times

    _CACHE["runner"] = run
    return run


def _prepare(edge_index):
    if "pre" not in _CACHE:
        pre = _preprocess(np.asarray(edge_index))
        _CACHE["pre"] = pre
        import sys
        print(f"[kernel] pad ratios: L1 {pre['stats']['pad1']:.3f} "
              f"L2 {pre['stats']['pad2']:.3f}", file=sys.stderr)
    import os
    if "nc" not in _CACHE and not os.environ.get("GAT_NO_BUILD"):
        _CACHE["nc"] = _build_nc(_CACHE["pre"])
    return _CACHE["pre"]


def kernel(x, edge_index, W1, a_src1, a_dst1, b1, W2, a_src2, a_dst2, b2,
           n_timed=0):
    pre = _prepare(edge_index)
    in_maps = _make_inputs(np.asarray(x), np.asarray(edge_index),
                           np.asarray(W1), np.asarray(a_src1),
                           np.asarray(a_dst1), np.asarray(b1),
                           np.asarray(W2), np.asarray(a_src2),
                           np.asarray(a_dst2), np.asarray(b2), pre)
    run = _get_runner(pre)
    result, times = run(in_maps, n_timed=n_timed)
    slices = result["out"]                      # [NCORES, S, HID]
    out = np.empty((N, HID), np.float32)
    core_of, pos2 = pre["core_of"], pre["pos2"]
    # device stored row r = p*T + t for slot t*128+p
    sl = np.arange(S)
    rowmap = (sl % P) * T + sl // P
    out[np.arange(N)] = slices[core_of, rowmap[pos2]]
    if n_timed:
        kernel.last_times = times
    return out


kernel.last_times = []
